# revision 1
# baseline (speedup 1.0000x reference)
"""Trainium2 Bass kernel for nn_Encoding (vq_codebook).

Math (per batch b):
    xf = x[b].reshape(C, N).T                      # (N tokens, C)
    sl2[n,k] = scale[k] * (|xf_n|^2 - 2 xf_n.c_k + |c_k|^2)
    w = softmax_k(sl2)                             # max-subtract skipped: sl2 in (-600, -0.18]
    out[b] = w.T @ xf - (sum_n w)[:,None] * codewords

Sharding: data-parallel over batch B=32 -> 4 batches per core on 8 cores.

Per-core dataflow (unit = 2048 tokens; 2 units/batch, 8 units/core):
  - x loaded in natural (c-partition, token-free) layout, 1 MiB DMAs.
  - PE is_transpose matmuls build xT (token-partition) tiles in PSUM; DVE/ACT
    evacuate them to SBUF for mm2 while a fused square+reduce (DVE
    tensor_tensor_reduce / ACT activation(Square, accum_out)) produces exact
    fp32 per-token |x|^2 columns.
  - mm1: psum_sl2 (128 = 4 groups x 32 codes, 512 tokens) accumulates
    A = -2*scale*cw against streamed x (fp32r, 1 cyc/row), one 32-col group
    per 512-token group.
  - |x|^2 columns are transposed (PE) and bounced through DRAM to become
    (4, 512) rows; a rank-4 fp32 matmul adds scale_k * |x|^2 into the same
    PSUM (full fp32 accuracy where it matters).
  - One ACT exp over (128, 512) with per-partition bias scale_k*|c_k|^2.
  - Softmax denominators: PE matmul with group-indicator lhsT -> (4, 512);
    DVE reciprocal; PE matmul broadcasts reciprocals back to (128, 512);
    DVE multiply normalizes -> w.
  - PE transposes w into (token, code) tiles; mm2 (w stationary, xT moving,
    fp32r) accumulates out (32, 256) per batch; wsum rides the same PSUM bank
    via a negated-identity matmul of DVE row-sums of w.
  - Final: one DVE scalar_tensor_tensor: out = cw*(-wsum) + wx; DMA out.
"""

import numpy as np
from contextlib import ExitStack

import concourse.bass as bass
import concourse.bacc as bacc
import concourse.mybir as mybir
import concourse.tile as tile
from concourse.bass_utils import run_bass_kernel_spmd

F32 = mybir.dt.float32
F32R = mybir.dt.float32r
ALU = mybir.AluOpType
ACTF = mybir.ActivationFunctionType

N_CORES = 8
B, C, K = 32, 256, 32
HW = 64 * 64            # 4096 tokens per batch
BL = B // N_CORES       # batches per core
UNIT = 2048             # tokens per unit
UNITS = BL * HW // UNIT  # 8 units per core
NGRP = 4                # 512-token groups per unit
GTOK = 512              # tokens per group
NCHUNK = 16             # 128-token chunks per unit


def build_module(bl=BL, debug=False):
    nc = bacc.Bacc(None)
    units = bl * HW // UNIT
    if debug:
        dbg_xsq4 = nc.dram_tensor("dbg_xsq4", (4, 512), F32, kind="ExternalOutput")
        dbg_e = nc.dram_tensor("dbg_e", (128, 512), F32, kind="ExternalOutput")
        dbg_wt = nc.dram_tensor("dbg_wt", (128, 512), F32, kind="ExternalOutput")
        dbg_xT = nc.dram_tensor("dbg_xT", (128, 16 * 258), F32, kind="ExternalOutput")
        dbg_xsqT = nc.dram_tensor("dbg_xsqT", (128, 16), F32, kind="ExternalOutput")
        dbg_wtT = nc.dram_tensor("dbg_wtT", (128, 512), F32, kind="ExternalOutput")
        dbg_pwx = nc.dram_tensor("dbg_pwx", (32, 257), F32, kind="ExternalOutput")

    x_d = nc.dram_tensor("x", (bl, 2, 128, HW), F32R, kind="ExternalInput")
    a_d = nc.dram_tensor("A", (2, 4, 128, 128), F32R, kind="ExternalInput")
    scbd_d = nc.dram_tensor("SCBD", (4, 128), F32, kind="ExternalInput")
    bias_d = nc.dram_tensor("BIASB", (128, 1), F32, kind="ExternalInput")
    gs_d = nc.dram_tensor("GS", (128, 4), F32R, kind="ExternalInput")
    gb_d = nc.dram_tensor("GB", (4, 128), F32, kind="ExternalInput")
    cw_d = nc.dram_tensor("CWD", (32, 256), F32, kind="ExternalInput")
    onz_d = nc.dram_tensor("ONZ", (128, 32), F32, kind="ExternalInput")
    idt_d = nc.dram_tensor("IDT", (128, 128), F32, kind="ExternalInput")
    out_d = nc.dram_tensor("out", (bl, 32, 256), F32, kind="ExternalOutput")

    with tile.TileContext(nc) as tc, ExitStack() as ctx:
        sb = ctx.enter_context(tc.tile_pool(name="sb", bufs=2))
        cp = ctx.enter_context(tc.tile_pool(name="consts", bufs=1))
        ps_big = ctx.enter_context(tc.tile_pool(name="ps_big", bufs=2, space="PSUM"))
        ps_sm = ctx.enter_context(tc.tile_pool(name="ps_sm", bufs=2, space="PSUM"))
        ps_xt = ctx.enter_context(tc.tile_pool(name="ps_xt", bufs=2, space="PSUM"))
        ps_wtt = ctx.enter_context(tc.tile_pool(name="ps_wtt", bufs=1, space="PSUM"))
        ps_wx = ctx.enter_context(tc.tile_pool(name="ps_wx", bufs=1, space="PSUM"))
        dr = ctx.enter_context(tc.tile_pool(name="dr", bufs=2, space="DRAM"))

        def c(shape, dram, tag, dt=F32):
            t = cp.tile(shape, dt, tag=tag)
            nc.sync.dma_start(t[:], dram[:])
            return t

        a_s = cp.tile([128, 8, 128], F32R, tag="a")
        nc.sync.dma_start(a_s[:], a_d[:].rearrange("c g p m -> p (c g) m"))
        scbd_s = c([4, 128], scbd_d, "scbd")
        bias_s = c([128, 1], bias_d, "bias")
        gs_s = c([128, 4], gs_d, "gs", F32R)
        gb_s = c([4, 128], gb_d, "gb")
        cw_s = c([32, 256], cw_d, "cw")
        idt_s = c([128, 128], idt_d, "idt")
        onz_s = c([128, 32], onz_d, "onz")

        pwx = {}

        def stage_a(u):
            """Load x, build xT + |x|^2, run mm1 (+xsq fold) into psum_sl2."""
            b_, uu = u // 2, u % 2
            t0 = uu * UNIT
            xn = sb.tile([128, 2 * UNIT], F32R, tag="xn")
            nc.sync.dma_start(xn[:, 0:UNIT], x_d[b_, 0, :, t0:t0 + UNIT])
            nc.sync.dma_start(xn[:, UNIT:2 * UNIT], x_d[b_, 1, :, t0:t0 + UNIT])

            xT = sb.tile([128, NCHUNK * 258], F32R, tag="xT")
            # per chunk: col 256 = ones (mm2 col 256 accumulates wsum),
            # col 257 = zeros (pad to even moving-dim for fp32r matmul).
            nc.vector.tensor_copy(
                xT[:].rearrange("p (j c) -> p j c", c=258)[:, :, 256:258],
                onz_s[:].rearrange("p (j c) -> p j c", c=2))
            xsqT = sb.tile([128, NCHUNK], F32, tag="xsqT")
            bno = sb.tile([128, NCHUNK // 2, 6], F32, tag="bno")
            for j2 in range(NCHUNK // 2):
                xtp = ps_xt.tile([128, 512], F32, tag="xt")
                for h in (0, 1):
                    j = 2 * j2 + h
                    for cc in (0, 1):
                        nc.tensor.transpose(
                            xtp[:, h * 256 + cc * 128:h * 256 + cc * 128 + 128],
                            xn[:, cc * UNIT + j * 128:cc * UNIT + j * 128 + 128].bitcast(F32),
                            idt_s[:],
                        )
                for h in (0, 1):
                    j = 2 * j2 + h
                    src = xtp[:, h * 256:(h + 1) * 256]
                    dst = xT[:, j * 258:j * 258 + 256]
                    if j % 2 == 0:
                        # ACT evacuates psum; DVE takes exact fp32 moments
                        # from psum (single psum read); |x|^2 reconstructed
                        # below from mean/var of even/odd element streams.
                        nc.scalar.copy(dst, src)
                        nc.vector.bn_stats(bno[:, j // 2, :], src)
                    else:
                        # DVE evacuates psum; ACT squares from psum.
                        sqj = sb.tile([128, 256], F32, tag="sqja")
                        nc.scalar.activation(
                            sqj[:], src, ACTF.Square,
                            accum_out=xsqT[:, j:j + 1],
                        )
                        nc.vector.tensor_copy(dst, src)

            # |x|^2 for even chunks: n*var_e + n*var_o + n*(mean_e^2+mean_o^2)
            t1 = sb.tile([128, NCHUNK // 2], F32, tag="t1")
            nc.vector.tensor_tensor(t1[:], bno[:, :, 1], bno[:, :, 1], ALU.mult)
            t2 = sb.tile([128, NCHUNK // 2], F32, tag="t2")
            nc.vector.tensor_tensor(t2[:], bno[:, :, 4], bno[:, :, 4], ALU.mult)
            s1 = sb.tile([128, NCHUNK // 2], F32, tag="s1")
            nc.vector.tensor_tensor(s1[:], bno[:, :, 2], bno[:, :, 5], ALU.add)
            s2 = sb.tile([128, NCHUNK // 2], F32, tag="s2")
            nc.vector.tensor_tensor(s2[:], t1[:], t2[:], ALU.add)
            xsqT_even = xsqT[:].rearrange("p (j two) -> p j two", two=2)[:, :, 0]
            nc.vector.scalar_tensor_tensor(
                out=xsqT_even, in0=s2[:], scalar=128.0, in1=s1[:],
                op0=ALU.mult, op1=ALU.add,
            )

            # crossing: xsqT (128,16) cols -> xsq4 (4,512) rows via PE
            # transpose + DRAM bounce (pure reshape).
            tsp = ps_sm.tile([16, 128], F32, tag="sm")
            nc.tensor.transpose(tsp[:], xsqT[:], idt_s[:])
            tss = sb.tile([16, 128], F32, tag="tss")
            nc.vector.tensor_copy(tss[:], tsp[:])
            drt = dr.tile([2048], F32, tag="drs")
            nc.scalar.dma_start(drt[:].rearrange("(j p) -> j p", j=16), tss[:])
            xsq4 = sb.tile([4, 512], F32, tag="xsq4")
            nc.scalar.dma_start(
                xsq4[:], drt[:].rearrange("(g t) -> g t", g=4))

            psl2 = ps_big.tile([128, 512], F32, tag="big")
            first = True
            for g in range(NGRP):
                for cc in (0, 1):
                    nc.tensor.matmul(
                        psl2[:, :],
                        a_s[:, cc * 4 + g, :],
                        xn[:, cc * UNIT + g * GTOK:cc * UNIT + (g + 1) * GTOK],
                        start=first, stop=False, skip_group_check=True,
                    )
                    first = False
            nc.tensor.matmul(
                psl2[:, :], scbd_s[:], xsq4[:],
                start=False, stop=True, skip_group_check=True,
            )
            if debug and u == 0:
                nc.scalar.dma_start(dbg_xsq4[:], xsq4[:])
                nc.scalar.dma_start(dbg_xT[:], xT[:].bitcast(F32))
                nc.scalar.dma_start(dbg_xsqT[:], xsqT[:])
            return dict(psl2=psl2, xT=xT, b=b_, uu=uu, u=u)

        def stage_b(st):
            """softmax + mm2 + (end of batch) final subtract + store."""
            psl2, xT, b_, uu = st["psl2"], st["xT"], st["b"], st["uu"]
            e = sb.tile([128, 512], F32R, tag="e")
            nc.scalar.activation(e[:], psl2[:], ACTF.Exp, bias=bias_s[:])
            ps4 = ps_sm.tile([4, 512], F32, tag="sm")
            nc.tensor.matmul(ps4[:], gs_s[:], e[:])
            r4 = sb.tile([4, 512], F32, tag="r4")
            nc.vector.reciprocal(r4[:], ps4[:])
            pR = ps_big.tile([128, 512], F32, tag="big")
            nc.tensor.matmul(pR[:], gb_s[:], r4[:])
            wt = sb.tile([128, 512], F32, tag="wt")
            nc.vector.tensor_tensor(wt[:], e[:].bitcast(F32), pR[:], ALU.mult)
            if debug and st["u"] == 0:
                nc.scalar.dma_start(dbg_e[:], e[:].bitcast(F32))
                nc.scalar.dma_start(dbg_wt[:], wt[:])

            if uu == 0:
                pwx[b_] = ps_wx.tile([32, 258], F32, tag="wx", name="pwx")

            pwtT = ps_wtt.tile([128, 512], F32, tag="wtt")
            for sl in range(4):
                # transpose of the full (128, 128) slice: column-block g of
                # the result is wT for token-chunk j = 4*g + sl.
                nc.tensor.transpose(
                    pwtT[:, 128 * sl:128 * sl + 128],
                    wt[:, 128 * sl:128 * sl + 128],
                    idt_s[:],
                )
            wtTs = sb.tile([128, 512], F32R, tag="wtTs")
            nc.vector.tensor_copy(wtTs[:], pwtT[:])
            if debug and st["u"] == 0:
                nc.scalar.dma_start(dbg_wtT[:], wtTs[:].bitcast(F32))
            for j in range(NCHUNK):
                nc.tensor.matmul(
                    pwx[b_][:, 0:258],
                    wtTs[:, 128 * (j % 4) + 32 * (j // 4):128 * (j % 4) + 32 * (j // 4) + 32],
                    xT[:, 258 * j:258 * j + 258],
                    start=(uu == 0 and j == 0), stop=(uu == 1 and j == NCHUNK - 1),
                    skip_group_check=True,
                )
            if uu == 1:
                if debug and b_ == 0:
                    pcp = sb.tile([32, 257], F32, tag="pcp")
                    nc.vector.tensor_copy(pcp[:], pwx[b_][:, 0:257])
                    nc.scalar.dma_start(dbg_pwx[:], pcp[:])
                outs = sb.tile([32, 256], F32, tag="outs")
                nc.vector.scalar_tensor_tensor(
                    out=outs[:], in0=cw_s[:], scalar=pwx[b_][:, 256:257],
                    in1=pwx[b_][:, 0:256], op0=ALU.mult, op1=ALU.add,
                )
                nc.scalar.dma_start(out_d[b_], outs[:])
                del pwx[b_]

        prev = stage_a(0)
        for u in range(1, units):
            cur = stage_a(u)
            stage_b(prev)
            prev = cur
        stage_b(prev)

    nc.finalize()
    return nc


def host_constants(codewords, scale):
    cw = np.asarray(codewords, dtype=np.float32)
    sc = np.asarray(scale, dtype=np.float32)
    c_sq = (cw.astype(np.float64) ** 2).sum(-1).astype(np.float32)

    A = np.zeros((2, 4, 128, 128), np.float32)
    for cc in range(2):
        blk = (-2.0 * sc[None, :]) * cw[:, cc * 128:(cc + 1) * 128].T
        for g in range(4):
            A[cc, g, :, 32 * g:32 * g + 32] = blk

    SCBD = np.zeros((4, 128), np.float32)
    BIASB = np.zeros((128, 1), np.float32)
    GS = np.zeros((128, 4), np.float32)
    GB = np.zeros((4, 128), np.float32)
    for g in range(4):
        SCBD[g, 32 * g:32 * g + 32] = sc
        BIASB[32 * g:32 * g + 32, 0] = sc * c_sq
        GS[32 * g:32 * g + 32, g] = 1.0
        GB[g, 32 * g:32 * g + 32] = 1.0

    return {
        "A": A, "SCBD": SCBD, "BIASB": BIASB, "GS": GS, "GB": GB,
        "CWD": np.ascontiguousarray(-cw),
        "ONZ": np.tile(np.array([1.0, 0.0], np.float32), (128, 16)),
        "IDT": np.eye(128, dtype=np.float32),
    }


_CACHE = {}


def kernel(x, codewords, scale):
    x = np.ascontiguousarray(np.asarray(x), dtype=np.float32)
    if "nc" not in _CACHE:
        _CACHE["nc"] = build_module()
    nc = _CACHE["nc"]
    consts = host_constants(codewords, scale)
    xs = x.reshape(B, 2, 128, HW)
    in_maps = []
    for i in range(N_CORES):
        m = dict(consts)
        m["x"] = np.ascontiguousarray(xs[BL * i:BL * (i + 1)])
        in_maps.append(m)
    res = run_bass_kernel_spmd(nc, in_maps, list(range(N_CORES)))
    out = np.concatenate([r["out"] for r in res.results], axis=0)
    return out.astype(np.float32)



# revision 40
# speedup vs baseline: 1.6095x; 1.6095x over previous
"""Trainium2 Bass kernel for nn_Encoding (vq_codebook).

Math (per batch b):
    xf = x[b].reshape(C, N).T                      # (N tokens, C)
    sl2[n,k] = scale[k] * (|xf_n|^2 - 2 xf_n.c_k + |c_k|^2)
    w = softmax_k(sl2)                             # max-subtract skipped: sl2 in (-600, -0.18]
    out[b] = w.T @ xf - (sum_n w)[:,None] * codewords

Sharding: data-parallel over batch B=32 -> 4 batches per core on 8 cores.
x is shipped to the device as bf16 (host cast): halves HBM traffic and
keeps rel err ~2e-3 against the 2e-2 gate (validated in fp64 emulation).

Per-core dataflow (unit = 2048 tokens; 2 units/batch, 8 units/core):
  - x loaded in natural (c-partition, token-free) bf16 layout, 512 KiB DMAs,
    prefetched one unit ahead.
  - |x|^2 entirely on PE: DVE squares xn into fp16 (2x mode, one unit ahead);
    ones-basis matmuls reduce over channels into a (4 group, 512 token) PSUM
    tile (fp32-exact); after evac, one rank-4 f32r matmul folds
    scale_k * |x|^2 into psl2.  No cross-layout shuffle needed.
  - PE is_transpose matmuls (bf16 identity -> 1 cyc/row) build xT tiles in
    bf16 PSUM; ACT/DVE/Pool evacuate them to SBUF for mm2.
  - mm1: psl2 (128 = 4 groups x 32 codes, 512 tokens) accumulates
    A = -2*scale*cw (bf16) against streamed bf16 x, one 32-col group per
    512-token group.
  - One ACT exp over (128, 512) with per-partition fp32 bias scale_k*|c_k|^2
    writes e as bf16.
  - Softmax denominators: PE matmul (bf16 group-indicator) -> (4, 512);
    DVE reciprocal; PE matmul broadcasts reciprocals back to (128, 512);
    DVE multiply normalizes -> w (bf16).
  - PE transposes w into (token, code) tiles (bf16 PSUM); DVE 2x-evacuates;
    mm2 (w stationary, xT moving, both bf16) accumulates out (32, 258) per
    batch; wsum rides col 256 via a ones-column in xT.
  - Final: one DVE scalar_tensor_tensor: out = cw*(-wsum) + wx; DMA out.
  - Unit u's softmax chain (exp..mm2) is interleaved into unit u+1's
    emission so each cross-engine hop overlaps transpose/mm1 work.
"""

import numpy as np
from contextlib import ExitStack

import ml_dtypes
import concourse.bass as bass
import concourse.bacc as bacc
import concourse.mybir as mybir
import concourse.tile as tile
from concourse.bass_utils import run_bass_kernel_spmd

F32 = mybir.dt.float32
F32R = mybir.dt.float32r
BF16 = mybir.dt.bfloat16
FP16 = mybir.dt.float16
ALU = mybir.AluOpType
ACTF = mybir.ActivationFunctionType

N_CORES = 8
B, C, K = 32, 256, 32
HW = 64 * 64            # 4096 tokens per batch
BL = B // N_CORES       # batches per core
UNIT = 2048             # tokens per unit
UNITS = BL * HW // UNIT  # 8 units per core
NCHUNK = 16             # 128-token chunks per unit
XTW = 258               # xT cols per chunk: 256 data + ones + pad


def build_module(bl=BL, debug=False):
    nc = bacc.Bacc(None)
    units = bl * HW // UNIT
    if debug:
        dbg_xT = nc.dram_tensor("dbg_xT", (128, NCHUNK * XTW), BF16, kind="ExternalOutput")
        dbg_q4 = nc.dram_tensor("dbg_q4", (4, 512), F32, kind="ExternalOutput")
        dbg_e = nc.dram_tensor("dbg_e", (128, 512), BF16, kind="ExternalOutput")
        dbg_wt = nc.dram_tensor("dbg_wt", (128, 512), BF16, kind="ExternalOutput")
        dbg_wtT = nc.dram_tensor("dbg_wtT", (128, 512), BF16, kind="ExternalOutput")

    x_d = nc.dram_tensor("x", (bl, 2, 128, HW), BF16, kind="ExternalInput")
    a_d = nc.dram_tensor("A", (2, 4, 128, 128), BF16, kind="ExternalInput")
    onb_d = nc.dram_tensor("ONB", (4, 128, 4), FP16, kind="ExternalInput")
    scbd_d = nc.dram_tensor("SCBD", (4, 128), F32R, kind="ExternalInput")
    bias_d = nc.dram_tensor("BIASB", (128, 1), F32, kind="ExternalInput")
    gs_d = nc.dram_tensor("GS", (128, 4), BF16, kind="ExternalInput")
    gb_d = nc.dram_tensor("GB", (4, 128), BF16, kind="ExternalInput")
    cw_d = nc.dram_tensor("CWD", (32, 256), F32, kind="ExternalInput")
    onz_d = nc.dram_tensor("ONZ", (128, 32), BF16, kind="ExternalInput")
    idt_d = nc.dram_tensor("IDT", (128, 128), BF16, kind="ExternalInput")
    out_d = nc.dram_tensor("out", (bl, 32, 256), F32, kind="ExternalOutput")

    with tile.TileContext(nc) as tc, ExitStack() as ctx:
        sb = ctx.enter_context(tc.tile_pool(name="sb", bufs=2))
        sbx = ctx.enter_context(tc.tile_pool(name="sbx", bufs=3))
        cp = ctx.enter_context(tc.tile_pool(name="consts", bufs=1))
        ps_xt = ctx.enter_context(tc.tile_pool(name="ps_xt", bufs=2, space="PSUM"))
        ps_big = ctx.enter_context(tc.tile_pool(name="ps_big", bufs=2, space="PSUM"))
        ps_d = ctx.enter_context(tc.tile_pool(name="ps_d", bufs=1, space="PSUM"))
        ps_q = ctx.enter_context(tc.tile_pool(name="ps_q", bufs=1, space="PSUM"))
        ps_wtt = ctx.enter_context(tc.tile_pool(name="ps_wtt", bufs=1, space="PSUM"))
        ps_wx = ctx.enter_context(tc.tile_pool(name="ps_wx", bufs=1, space="PSUM"))

        def c(shape, dram, tag, dt):
            t = cp.tile(shape, dt, tag=tag)
            nc.sync.dma_start(t[:], dram[:])
            return t

        a_s = cp.tile([128, 8, 128], BF16, tag="a")
        nc.sync.dma_start(a_s[:], a_d[:].rearrange("c g p m -> p (c g) m"))
        onb_s = cp.tile([128, 4, 4], FP16, tag="onb")
        nc.sync.dma_start(onb_s[:], onb_d[:].rearrange("g p m -> p g m"))
        scbd_s = c([4, 128], scbd_d, "scbd", F32R)
        bias_s = c([128, 1], bias_d, "bias", F32)
        gs_s = c([128, 4], gs_d, "gs", BF16)
        gb_s = c([4, 128], gb_d, "gb", BF16)
        cw_s = c([32, 256], cw_d, "cw", F32)
        idt_s = c([128, 128], idt_d, "idt", BF16)
        onz_s = c([128, 32], onz_d, "onz", BF16)

        pwx = {}
        pending_out = []

        def load_xn(u):
            """Load x natural, fill xT's ones columns, and square xn for the
            PE |x|^2 reduction.  All run one unit ahead of stage(u)."""
            b_, uu = u // 2, u % 2
            t0 = uu * UNIT
            xn = sbx.tile([128, 2, UNIT], BF16, tag="xn")
            nc.sync.dma_start(xn[:, 0], x_d[b_, 0, :, t0:t0 + UNIT])
            nc.sync.dma_start(xn[:, 1], x_d[b_, 1, :, t0:t0 + UNIT])
            xT = sbx.tile([128, NCHUNK * XTW], BF16, tag="xT")
            xTv = xT[:].rearrange("p (j c) -> p j c", c=XTW)
            # col 256 = ones (mm2 col 256 accumulates wsum), col 257 = pad.
            nc.gpsimd.tensor_copy(
                xTv[:, :, 256:258],
                onz_s[:].rearrange("p (j c) -> p j c", c=2))
            xq = sbx.tile([128, 2, UNIT], FP16, tag="xq")
            nc.vector.tensor_tensor(xq[:, 0], xn[:, 0], xn[:, 0], ALU.mult)
            nc.vector.tensor_tensor(xq[:, 1], xn[:, 1], xn[:, 1], ALU.mult)
            return xn, xT, xq

        def stage(u, prev, xn, xT, xq):
            """Emit A(u) interleaved with B(prev)."""
            b_, uu = u // 2, u % 2

            xTv = xT[:].rearrange("p (j c) -> p j c", c=XTW)
            st = dict(xT=xT, b=b_, uu=uu, u=u)
            psl2 = ps_big.tile([128, 512], F32, tag="big")
            st["psl2"] = psl2

            def mm1_part(i):
                g, cc = divmod(i, 2)
                nc.tensor.matmul(
                    psl2[:, :],
                    a_s[:, cc * 4 + g, :],
                    xn[:, cc, g * 512:(g + 1) * 512],
                    start=(i == 0), stop=False, skip_group_check=True,
                )

            def q_part(q4, i):
                # q4[g, n'] += sum_c xq[c, 512g + n']  (exact fp32)
                g, cc = divmod(i, 2)
                nc.tensor.matmul(
                    q4[:, :],
                    onb_s[:, g, :],
                    xq[:, cc, g * 512:(g + 1) * 512],
                    start=(i == 0), stop=(i == 7), skip_group_check=True,
                )

            def tgroup(j2):
                # PE transposes for both cc halves of 2 chunks
                xtp = ps_xt.tile([128, 512], BF16, tag="xt")
                for h in (0, 1):
                    j = 2 * j2 + h
                    for cc in (0, 1):
                        nc.tensor.transpose(
                            xtp[:, h * 256 + cc * 128:h * 256 + cc * 128 + 128],
                            xn[:, cc, j * 128:j * 128 + 128],
                            idt_s[:],
                        )
                # evacuate both chunks in one strided op
                dst = xTv[:, 2 * j2:2 * j2 + 2, 0:256]
                src = xtp[:].rearrange("p (h c) -> p h c", c=256)
                if j2 in (0, 2, 3, 5, 6):
                    nc.scalar.copy(dst, src)
                else:
                    nc.vector.tensor_copy(dst, src)

            p = prev  # may be None (first unit)

            # emit deferred batch-output stores: by now the STT that feeds
            # them has drained, so the DMA doesn't block the ACT sequencer.
            while pending_out:
                ob, outs = pending_out.pop(0)
                nc.scalar.dma_start(out_d[ob], outs[:])

            # |x|^2 channel-reduction: 8 accumulating matmuls, exact fp32
            q4 = ps_q.tile([4, 512], F32, tag="q")
            for i in range(8):
                q_part(q4, i)
            tgroup(0)
            if p is not None:
                e = sb.tile([128, 512], BF16, tag="e")
                nc.scalar.activation(e[:], p["psl2"][:], ACTF.Exp, bias=bias_s[:])
            tgroup(1)
            mm1_part(0)
            mm1_part(1)
            if p is not None:
                ps4 = ps_d.tile([4, 512], F32, tag="d")
                nc.tensor.matmul(ps4[:], gs_s[:], e[:])
            tgroup(2)
            mm1_part(2)
            if p is not None:
                r4 = sb.tile([4, 512], BF16, tag="r4")
                with nc.allow_low_precision(reason="1/d in bf16: per-token scale, cancels in out"):
                    nc.vector.reciprocal(r4[:], ps4[:])
            tgroup(3)
            mm1_part(3)
            if p is not None:
                pR = ps_big.tile([128, 512], F32, tag="big")
                nc.tensor.matmul(pR[:], gb_s[:], r4[:])
            tgroup(4)
            mm1_part(4)
            if p is not None:
                wt = sb.tile([128, 512], BF16, tag="wt")
                nc.vector.tensor_tensor(wt[:], e[:], pR[:], ALU.mult)
            tgroup(5)
            mm1_part(5)
            # evacuate |x|^2 row-block for the fold matmul
            q4s = sb.tile([4, 512], F32R, tag="q4s")
            nc.scalar.copy(q4s[:], q4[:].bitcast(F32R))
            tgroup(6)
            mm1_part(6)
            if p is not None:
                if debug and p["u"] == 0:
                    nc.scalar.dma_start(dbg_xT[:], p["xT"][:])
                    nc.scalar.dma_start(dbg_e[:], e[:])
                    nc.scalar.dma_start(dbg_wt[:], wt[:])
                pwtT = ps_wtt.tile([128, 512], BF16, tag="wtt")
                for sl in range(4):
                    # transpose of the (128, 128) slice: column-block g of
                    # the result is wT for token-chunk j = 4*g + sl.
                    nc.tensor.transpose(
                        pwtT[:, 128 * sl:128 * sl + 128],
                        wt[:, 128 * sl:128 * sl + 128],
                        idt_s[:],
                    )
            tgroup(7)
            mm1_part(7)
            if p is not None:
                wtTs = sb.tile([128, 512], BF16, tag="wtTs")
                nc.vector.tensor_copy(wtTs[:], pwtT[:])
                if debug and p["u"] == 0:
                    nc.scalar.dma_start(dbg_wtT[:], wtTs[:])
            # fold scale_k * |x|^2 into psl2 and close the accumulation
            if debug and u == 0:
                nc.scalar.dma_start(dbg_q4[:], q4s[:].bitcast(F32))
            nc.tensor.matmul(
                psl2[:, :], scbd_s[:], q4s[:],
                start=False, stop=True, skip_group_check=True,
            )
            if p is not None:
                emit_mm2(p, wtTs)
            # tail: prefetch + xbar + squares for u+1
            if u + 1 < units:
                nxt = load_xn(u + 1)
            else:
                nxt = (None, None, None)
            return st, nxt

        def emit_mm2(p, wtTs):
            b_, uu, xT = p["b"], p["uu"], p["xT"]
            if uu == 0:
                pwx[b_] = ps_wx.tile([32, XTW], F32, tag="wx", name="pwx")
            for j in range(NCHUNK):
                nc.tensor.matmul(
                    pwx[b_][:, 0:XTW],
                    wtTs[:, 128 * (j % 4) + 32 * (j // 4):128 * (j % 4) + 32 * (j // 4) + 32],
                    xT[:, XTW * j:XTW * j + XTW],
                    start=(uu == 0 and j == 0), stop=(uu == 1 and j == NCHUNK - 1),
                    skip_group_check=True,
                )
            if uu == 1:
                outs = sb.tile([32, 256], F32, tag="outs")
                nc.vector.scalar_tensor_tensor(
                    out=outs[:], in0=cw_s[:], scalar=pwx[b_][:, 256:257],
                    in1=pwx[b_][:, 0:256], op0=ALU.mult, op1=ALU.add,
                )
                pending_out.append((b_, outs))
                del pwx[b_]

        cur = load_xn(0)
        prev = None
        for u in range(units):
            prev, cur = stage(u, prev, *cur)
        # epilogue: drain the last unit's softmax chain
        e = sb.tile([128, 512], BF16, tag="e")
        nc.scalar.activation(e[:], prev["psl2"][:], ACTF.Exp, bias=bias_s[:])
        ps4 = ps_d.tile([4, 512], F32, tag="d")
        nc.tensor.matmul(ps4[:], gs_s[:], e[:])
        r4 = sb.tile([4, 512], BF16, tag="r4")
        with nc.allow_low_precision(reason="1/d in bf16: per-token scale, cancels in out"):
            nc.vector.reciprocal(r4[:], ps4[:])
        pR = ps_big.tile([128, 512], F32, tag="big")
        nc.tensor.matmul(pR[:], gb_s[:], r4[:])
        wt = sb.tile([128, 512], BF16, tag="wt")
        nc.vector.tensor_tensor(wt[:], e[:], pR[:], ALU.mult)
        pwtT = ps_wtt.tile([128, 512], BF16, tag="wtt")
        for sl in range(4):
            nc.tensor.transpose(
                pwtT[:, 128 * sl:128 * sl + 128],
                wt[:, 128 * sl:128 * sl + 128],
                idt_s[:],
            )
        wtTs = sb.tile([128, 512], BF16, tag="wtTs")
        nc.vector.tensor_copy(wtTs[:], pwtT[:])
        emit_mm2(prev, wtTs)
        while pending_out:
            ob, outs = pending_out.pop(0)
            nc.scalar.dma_start(out_d[ob], outs[:])

    nc.finalize()
    return nc


def host_constants(codewords, scale):
    cw = np.asarray(codewords, dtype=np.float32)
    sc = np.asarray(scale, dtype=np.float32)
    c_sq = (cw.astype(np.float64) ** 2).sum(-1).astype(np.float32)

    A = np.zeros((2, 4, 128, 128), np.float32)
    for cc in range(2):
        blk = (-2.0 * sc[None, :]) * cw[:, cc * 128:(cc + 1) * 128].T
        for g in range(4):
            A[cc, g, :, 32 * g:32 * g + 32] = blk

    ONB = np.zeros((4, 128, 4), np.float32)
    for g in range(4):
        ONB[g, :, g] = 1.0

    SCBD = np.zeros((4, 128), np.float32)
    BIASB = np.zeros((128, 1), np.float32)
    GS = np.zeros((128, 4), np.float32)
    GB = np.zeros((4, 128), np.float32)
    for g in range(4):
        SCBD[g, 32 * g:32 * g + 32] = sc
        BIASB[32 * g:32 * g + 32, 0] = sc * c_sq
        GS[32 * g:32 * g + 32, g] = 1.0
        GB[g, 32 * g:32 * g + 32] = 1.0

    bf = ml_dtypes.bfloat16
    return {
        "A": A.astype(bf), "ONB": ONB.astype(np.float16), "SCBD": SCBD,
        "BIASB": BIASB, "GS": GS.astype(bf), "GB": GB.astype(bf),
        "CWD": np.ascontiguousarray(-cw),
        "ONZ": np.tile(np.array([1.0, 0.0], bf), (128, 16)),
        "IDT": np.eye(128, dtype=bf),
    }


def make_in_maps(x, codewords, scale):
    consts = host_constants(codewords, scale)
    xb = np.asarray(x).astype(ml_dtypes.bfloat16)
    xs = xb.reshape(B, 2, 128, HW)
    in_maps = []
    for i in range(N_CORES):
        m = dict(consts)
        m["x"] = np.ascontiguousarray(xs[BL * i:BL * (i + 1)])
        in_maps.append(m)
    return in_maps


_CACHE = {}


def kernel(x, codewords, scale):
    if "nc" not in _CACHE:
        _CACHE["nc"] = build_module()
    nc = _CACHE["nc"]
    in_maps = make_in_maps(x, codewords, scale)
    res = run_bass_kernel_spmd(nc, in_maps, list(range(N_CORES)))
    out = np.concatenate([r["out"] for r in res.results], axis=0)
    return out.astype(np.float32)


# revision 51
# speedup vs baseline: 68617.0044x; 42633.0870x over previous
"""Trainium2 Bass kernel for nn_Encoding (vq_codebook).

Math (per batch b):
    xf = x[b].reshape(C, N).T                      # (N tokens, C)
    sl2[n,k] = scale[k] * (|xf_n|^2 - 2 xf_n.c_k + |c_k|^2)
    w = softmax_k(sl2)                             # max-subtract skipped: sl2 in (-600, -0.18]
    out[b] = w.T @ xf - (sum_n w)[:,None] * codewords

Sharding: data-parallel over batch B=32 -> 4 batches per core on 8 cores.
x is shipped to the device as bf16 (host cast): halves HBM traffic and
keeps rel err ~2e-3 against the 2e-2 gate (validated in fp64 emulation).

Per-core dataflow (unit = 2048 tokens; 2 units/batch, 8 units/core):
  - x loaded in natural (c-partition, token-free) bf16 layout, 512 KiB DMAs,
    prefetched one unit ahead.
  - |x|^2 entirely on PE: DVE squares xn into fp16 (2x mode, one unit ahead);
    ones-basis matmuls reduce over channels into a (4 group, 512 token) PSUM
    tile (fp32-exact); after evac, one rank-4 f32r matmul folds
    scale_k * |x|^2 into psl2.  No cross-layout shuffle needed.
  - PE is_transpose matmuls (bf16 identity -> 1 cyc/row) build xT tiles in
    bf16 PSUM; ACT/DVE/Pool evacuate them to SBUF for mm2.
  - mm1: psl2 (128 = 4 groups x 32 codes, 512 tokens) accumulates
    A = -2*scale*cw (bf16) against streamed bf16 x, one 32-col group per
    512-token group.
  - One ACT exp over (128, 512) with per-partition fp32 bias scale_k*|c_k|^2
    writes e as bf16.
  - Softmax denominators: PE matmul (bf16 group-indicator) -> (4, 512);
    DVE reciprocal; PE matmul broadcasts reciprocals back to (128, 512);
    DVE multiply normalizes -> w (bf16).
  - PE transposes w into (token, code) tiles (bf16 PSUM); DVE 2x-evacuates;
    mm2 (w stationary, xT moving, both bf16) accumulates out (32, 258) per
    batch; wsum rides col 256 via a ones-column in xT.
  - Final: one DVE scalar_tensor_tensor: out = cw*(-wsum) + wx; DMA out.
  - Unit u's softmax chain (exp..mm2) is interleaved into unit u+1's
    emission so each cross-engine hop overlaps transpose/mm1 work.
"""

import numpy as np
from contextlib import ExitStack

import ml_dtypes
import concourse.bass as bass
import concourse.bacc as bacc
import concourse.mybir as mybir
import concourse.tile as tile
from concourse.bass_utils import run_bass_kernel_spmd

F32 = mybir.dt.float32
F32R = mybir.dt.float32r
BF16 = mybir.dt.bfloat16
FP16 = mybir.dt.float16
ALU = mybir.AluOpType
ACTF = mybir.ActivationFunctionType

N_CORES = 8
B, C, K = 32, 256, 32
HW = 64 * 64            # 4096 tokens per batch
BL = B // N_CORES       # batches per core
UNIT = 2048             # tokens per unit
UNITS = BL * HW // UNIT  # 8 units per core
NCHUNK = 16             # 128-token chunks per unit
XTW = 258               # xT cols per chunk: 256 data + ones + pad


def build_module(bl=BL, debug=False):
    nc = bacc.Bacc(None)
    units = bl * HW // UNIT
    if debug:
        dbg_xT = nc.dram_tensor("dbg_xT", (128, NCHUNK * XTW), BF16, kind="ExternalOutput")
        dbg_q4 = nc.dram_tensor("dbg_q4", (4, 512), F32, kind="ExternalOutput")
        dbg_e = nc.dram_tensor("dbg_e", (128, 512), BF16, kind="ExternalOutput")
        dbg_wt = nc.dram_tensor("dbg_wt", (128, 512), BF16, kind="ExternalOutput")
        dbg_wtT = nc.dram_tensor("dbg_wtT", (128, 512), BF16, kind="ExternalOutput")

    x_d = nc.dram_tensor("x", (bl, 2, 128, HW), BF16, kind="ExternalInput")
    a_d = nc.dram_tensor("A", (128, 8, 128), BF16, kind="ExternalInput")
    onb_d = nc.dram_tensor("ONB", (128, 4, 4), FP16, kind="ExternalInput")
    scbd_d = nc.dram_tensor("SCBD", (4, 128), F32R, kind="ExternalInput")
    bias_d = nc.dram_tensor("BIASB", (128, 1), F32, kind="ExternalInput")
    gs_d = nc.dram_tensor("GS", (128, 4), BF16, kind="ExternalInput")
    gb_d = nc.dram_tensor("GB", (4, 128), BF16, kind="ExternalInput")
    cw_d = nc.dram_tensor("CWD", (32, 256), F32, kind="ExternalInput")
    onz_d = nc.dram_tensor("ONZ", (128, 32), BF16, kind="ExternalInput")
    idt_d = nc.dram_tensor("IDT", (128, 128), BF16, kind="ExternalInput")
    out_d = nc.dram_tensor("out", (bl, 32, 256), F32, kind="ExternalOutput")

    with tile.TileContext(nc) as tc, ExitStack() as ctx:
        sb = ctx.enter_context(tc.tile_pool(name="sb", bufs=2))
        sbx = ctx.enter_context(tc.tile_pool(name="sbx", bufs=3))
        cp = ctx.enter_context(tc.tile_pool(name="consts", bufs=1))
        ps_xt = ctx.enter_context(tc.tile_pool(name="ps_xt", bufs=2, space="PSUM"))
        ps_big = ctx.enter_context(tc.tile_pool(name="ps_big", bufs=2, space="PSUM"))
        ps_d = ctx.enter_context(tc.tile_pool(name="ps_d", bufs=1, space="PSUM"))
        ps_q = ctx.enter_context(tc.tile_pool(name="ps_q", bufs=1, space="PSUM"))
        ps_wtt = ctx.enter_context(tc.tile_pool(name="ps_wtt", bufs=1, space="PSUM"))
        ps_wx = ctx.enter_context(tc.tile_pool(name="ps_wx", bufs=1, space="PSUM"))

        def c(shape, dram, tag, dt):
            t = cp.tile(shape, dt, tag=tag)
            nc.sync.dma_start(t[:], dram[:])
            return t

        # the two tiny consts the first transposes/evacs need, then the
        # unit-0 x loads (they gate the pipeline), then the other consts.
        idt_s = c([128, 128], idt_d, "idt", BF16)
        onz_s = c([128, 32], onz_d, "onz", BF16)
        xn0 = sbx.tile([128, 2, UNIT], BF16, tag="xn")
        nc.sync.dma_start(xn0[:, 0], x_d[0, 0, :, 0:UNIT])
        nc.sync.dma_start(xn0[:, 1], x_d[0, 1, :, 0:UNIT])

        scbd_s = c([4, 128], scbd_d, "scbd", F32R)
        bias_s = c([128, 1], bias_d, "bias", F32)
        gs_s = c([128, 4], gs_d, "gs", BF16)
        gb_s = c([4, 128], gb_d, "gb", BF16)
        cw_s = c([32, 256], cw_d, "cw", F32)
        onb_s = cp.tile([128, 4, 4], FP16, tag="onb")
        nc.sync.dma_start(onb_s[:], onb_d[:])
        a_s = cp.tile([128, 8, 128], BF16, tag="a")
        nc.sync.dma_start(a_s[:], a_d[:])

        pwx = {}
        pending_out = []

        def load_xn(u, xn=None):
            """Load x natural, fill xT's ones columns, and square xn for the
            PE |x|^2 reduction.  All run one unit ahead of stage(u)."""
            b_, uu = u // 2, u % 2
            t0 = uu * UNIT
            if xn is None:
                xn = sbx.tile([128, 2, UNIT], BF16, tag="xn")
                nc.sync.dma_start(xn[:, 0], x_d[b_, 0, :, t0:t0 + UNIT])
                nc.sync.dma_start(xn[:, 1], x_d[b_, 1, :, t0:t0 + UNIT])
            xT = sbx.tile([128, NCHUNK * XTW], BF16, tag="xT")
            xTv = xT[:].rearrange("p (j c) -> p j c", c=XTW)
            # col 256 = ones (mm2 col 256 accumulates wsum), col 257 = pad.
            nc.gpsimd.tensor_copy(
                xTv[:, :, 256:258],
                onz_s[:].rearrange("p (j c) -> p j c", c=2))
            # xqs[c, n] = xn0[c,n]^2 + xn1[c,n]^2 (fp16, DVE 2x): halves
            # the PE channel-reduction matmuls for |x|^2.
            xq0 = sb.tile([128, UNIT], FP16, tag="xq0")
            nc.vector.tensor_tensor(xq0[:], xn[:, 0], xn[:, 0], ALU.mult)
            xq1 = sb.tile([128, UNIT], FP16, tag="xq1")
            nc.vector.tensor_tensor(xq1[:], xn[:, 1], xn[:, 1], ALU.mult)
            xq = sbx.tile([128, UNIT], FP16, tag="xq")
            nc.vector.tensor_tensor(xq[:], xq0[:], xq1[:], ALU.add)
            return xn, xT, xq

        def stage(u, prev, xn, xT, xq):
            """Emit A(u) interleaved with B(prev)."""
            b_, uu = u // 2, u % 2

            xTv = xT[:].rearrange("p (j c) -> p j c", c=XTW)
            st = dict(xT=xT, b=b_, uu=uu, u=u)
            psl2 = ps_big.tile([128, 512], F32, tag="big")
            st["psl2"] = psl2

            def mm1_part(i):
                g, cc = divmod(i, 2)
                nc.tensor.matmul(
                    psl2[:, :],
                    a_s[:, cc * 4 + g, :],
                    xn[:, cc, g * 512:(g + 1) * 512],
                    start=(i == 0), stop=False, skip_group_check=True,
                )

            def q_part(q4, g):
                # q4[g, n'] += sum_c xqs[c, 512g + n']  (exact fp32)
                nc.tensor.matmul(
                    q4[:, :],
                    onb_s[:, g, :],
                    xq[:, g * 512:(g + 1) * 512],
                    start=(g == 0), stop=(g == 3), skip_group_check=True,
                )

            def tgroup(j2):
                # PE transposes for both cc halves of 2 chunks
                xtp = ps_xt.tile([128, 512], BF16, tag="xt")
                for h in (0, 1):
                    j = 2 * j2 + h
                    for cc in (0, 1):
                        nc.tensor.transpose(
                            xtp[:, h * 256 + cc * 128:h * 256 + cc * 128 + 128],
                            xn[:, cc, j * 128:j * 128 + 128],
                            idt_s[:],
                        )
                # evacuate both chunks in one strided op
                dst = xTv[:, 2 * j2:2 * j2 + 2, 0:256]
                src = xtp[:].rearrange("p (h c) -> p h c", c=256)
                # unit 0: DVE is busy squaring x for the first |x|^2 pass,
                # so its evac tiles would stall the transpose rotation.
                if j2 in (0, 2, 3, 5, 6, 7) or p is None:
                    nc.scalar.copy(dst, src)
                else:
                    nc.vector.tensor_copy(dst, src)

            p = prev  # may be None (first unit)

            # emit deferred batch-output stores: by now the STT that feeds
            # them has drained, so the DMA doesn't block the ACT sequencer.
            while pending_out:
                ob, outs = pending_out.pop(0)
                nc.scalar.dma_start(out_d[ob], outs[:])

            # |x|^2 channel-reduction: 8 accumulating matmuls, exact fp32.
            # For unit 0 they are emitted late (xq(0) is still being computed
            # when PE starts; transposes/mm1 only need xn).
            q4 = ps_q.tile([4, 512], F32, tag="q")
            if p is not None:
                for i in range(4):
                    q_part(q4, i)
            tgroup(0)
            if p is not None:
                e = sb.tile([128, 512], BF16, tag="e")
                nc.scalar.activation(e[:], p["psl2"][:], ACTF.Exp, bias=bias_s[:])
            tgroup(1)
            mm1_part(0)
            mm1_part(1)
            if p is not None:
                ps4 = ps_d.tile([4, 512], F32, tag="d")
                nc.tensor.matmul(ps4[:], gs_s[:], e[:])
            tgroup(2)
            mm1_part(2)
            if p is not None:
                r4 = sb.tile([4, 512], BF16, tag="r4")
                with nc.allow_low_precision(reason="1/d in bf16: per-token scale, cancels in out"):
                    nc.vector.reciprocal(r4[:], ps4[:])
            tgroup(3)
            mm1_part(3)
            if p is not None:
                pR = ps_big.tile([128, 512], F32, tag="big")
                nc.tensor.matmul(pR[:], gb_s[:], r4[:])
            tgroup(4)
            mm1_part(4)
            if p is not None:
                wt = sb.tile([128, 512], BF16, tag="wt")
                nc.vector.tensor_tensor(wt[:], e[:], pR[:], ALU.mult)
            tgroup(5)
            mm1_part(5)
            if p is None:
                for i in range(4):
                    q_part(q4, i)
            # evacuate |x|^2 row-block for the fold matmul
            q4s = sb.tile([4, 512], F32R, tag="q4s")
            nc.scalar.copy(q4s[:], q4[:].bitcast(F32R))
            tgroup(6)
            mm1_part(6)
            if p is not None:
                if debug and p["u"] == 0:
                    nc.scalar.dma_start(dbg_xT[:], p["xT"][:])
                    nc.scalar.dma_start(dbg_e[:], e[:])
                    nc.scalar.dma_start(dbg_wt[:], wt[:])
                pwtT = ps_wtt.tile([128, 512], BF16, tag="wtt")
                for sl in range(4):
                    # transpose of the (128, 128) slice: column-block g of
                    # the result is wT for token-chunk j = 4*g + sl.
                    nc.tensor.transpose(
                        pwtT[:, 128 * sl:128 * sl + 128],
                        wt[:, 128 * sl:128 * sl + 128],
                        idt_s[:],
                    )
            tgroup(7)
            mm1_part(7)
            if p is not None:
                wtTs = sb.tile([128, 512], BF16, tag="wtTs")
                nc.vector.tensor_copy(wtTs[:], pwtT[:])
                if debug and p["u"] == 0:
                    nc.scalar.dma_start(dbg_wtT[:], wtTs[:])
            # fold scale_k * |x|^2 into psl2 and close the accumulation
            if debug and u == 0:
                nc.scalar.dma_start(dbg_q4[:], q4s[:].bitcast(F32))
            nc.tensor.matmul(
                psl2[:, :], scbd_s[:], q4s[:],
                start=False, stop=True, skip_group_check=True,
            )
            if p is not None:
                emit_mm2(p, wtTs)
            # tail: prefetch + xbar + squares for u+1
            if u + 1 < units:
                nxt = load_xn(u + 1)
            else:
                nxt = (None, None, None)
            return st, nxt

        def emit_mm2(p, wtTs):
            b_, uu, xT = p["b"], p["uu"], p["xT"]
            if uu == 0:
                pwx[b_] = ps_wx.tile([32, XTW], F32, tag="wx", name="pwx")
            for j in range(NCHUNK):
                nc.tensor.matmul(
                    pwx[b_][:, 0:XTW],
                    wtTs[:, 128 * (j % 4) + 32 * (j // 4):128 * (j % 4) + 32 * (j // 4) + 32],
                    xT[:, XTW * j:XTW * j + XTW],
                    start=(uu == 0 and j == 0), stop=(uu == 1 and j == NCHUNK - 1),
                    skip_group_check=True,
                )
            if uu == 1:
                outs = sb.tile([32, 256], F32, tag="outs")
                nc.vector.scalar_tensor_tensor(
                    out=outs[:], in0=cw_s[:], scalar=pwx[b_][:, 256:257],
                    in1=pwx[b_][:, 0:256], op0=ALU.mult, op1=ALU.add,
                )
                pending_out.append((b_, outs))
                del pwx[b_]

        cur = load_xn(0, xn=xn0)
        prev = None
        for u in range(units):
            prev, cur = stage(u, prev, *cur)
        # epilogue: drain the last unit's softmax chain
        e = sb.tile([128, 512], BF16, tag="e")
        nc.scalar.activation(e[:], prev["psl2"][:], ACTF.Exp, bias=bias_s[:])
        ps4 = ps_d.tile([4, 512], F32, tag="d")
        nc.tensor.matmul(ps4[:], gs_s[:], e[:])
        r4 = sb.tile([4, 512], BF16, tag="r4")
        with nc.allow_low_precision(reason="1/d in bf16: per-token scale, cancels in out"):
            nc.vector.reciprocal(r4[:], ps4[:])
        pR = ps_big.tile([128, 512], F32, tag="big")
        nc.tensor.matmul(pR[:], gb_s[:], r4[:])
        wt = sb.tile([128, 512], BF16, tag="wt")
        nc.vector.tensor_tensor(wt[:], e[:], pR[:], ALU.mult)
        pwtT = ps_wtt.tile([128, 512], BF16, tag="wtt")
        for sl in range(4):
            nc.tensor.transpose(
                pwtT[:, 128 * sl:128 * sl + 128],
                wt[:, 128 * sl:128 * sl + 128],
                idt_s[:],
            )
        wtTs = sb.tile([128, 512], BF16, tag="wtTs")
        nc.vector.tensor_copy(wtTs[:], pwtT[:])
        emit_mm2(prev, wtTs)
        while pending_out:
            ob, outs = pending_out.pop(0)
            nc.scalar.dma_start(out_d[ob], outs[:])

    nc.finalize()
    return nc


def host_constants(codewords, scale):
    cw = np.asarray(codewords, dtype=np.float32)
    sc = np.asarray(scale, dtype=np.float32)
    c_sq = (cw.astype(np.float64) ** 2).sum(-1).astype(np.float32)

    A = np.zeros((2, 4, 128, 128), np.float32)
    for cc in range(2):
        blk = (-2.0 * sc[None, :]) * cw[:, cc * 128:(cc + 1) * 128].T
        for g in range(4):
            A[cc, g, :, 32 * g:32 * g + 32] = blk

    ONB = np.zeros((128, 4, 4), np.float32)
    for g in range(4):
        ONB[:, g, g] = 1.0

    SCBD = np.zeros((4, 128), np.float32)
    BIASB = np.zeros((128, 1), np.float32)
    GS = np.zeros((128, 4), np.float32)
    GB = np.zeros((4, 128), np.float32)
    for g in range(4):
        SCBD[g, 32 * g:32 * g + 32] = sc
        BIASB[32 * g:32 * g + 32, 0] = sc * c_sq
        GS[32 * g:32 * g + 32, g] = 1.0
        GB[g, 32 * g:32 * g + 32] = 1.0

    bf = ml_dtypes.bfloat16
    return {
        "A": np.ascontiguousarray(A.transpose(2, 0, 1, 3).reshape(128, 8, 128)).astype(bf),
        "ONB": ONB.astype(np.float16), "SCBD": SCBD,
        "BIASB": BIASB, "GS": GS.astype(bf), "GB": GB.astype(bf),
        "CWD": np.ascontiguousarray(-cw),
        "ONZ": np.tile(np.array([1.0, 0.0], bf), (128, 16)),
        "IDT": np.eye(128, dtype=bf),
    }


def make_in_maps(x, codewords, scale):
    consts = host_constants(codewords, scale)
    xb = np.asarray(x).astype(ml_dtypes.bfloat16)
    xs = xb.reshape(B, 2, 128, HW)
    in_maps = []
    for i in range(N_CORES):
        m = dict(consts)
        m["x"] = np.ascontiguousarray(xs[BL * i:BL * (i + 1)])
        in_maps.append(m)
    return in_maps


_CACHE = {}


def kernel(x, codewords, scale):
    if "nc" not in _CACHE:
        _CACHE["nc"] = build_module()
    nc = _CACHE["nc"]
    in_maps = make_in_maps(x, codewords, scale)
    res = run_bass_kernel_spmd(nc, in_maps, list(range(N_CORES)))
    out = np.concatenate([r["out"] for r in res.results], axis=0)
    return out.astype(np.float32)


# revision 54
# speedup vs baseline: 69748.1007x; 1.0165x over previous
"""Trainium2 Bass kernel for nn_Encoding (vq_codebook).

Math (per batch b):
    xf = x[b].reshape(C, N).T                      # (N tokens, C)
    sl2[n,k] = scale[k] * (|xf_n|^2 - 2 xf_n.c_k + |c_k|^2)
    w = softmax_k(sl2)                             # max-subtract skipped: sl2 in (-600, -0.18]
    out[b] = w.T @ xf - (sum_n w)[:,None] * codewords

Sharding: data-parallel over batch B=32 -> 4 batches per core on 8 cores.
x is shipped to the device as bf16 (host cast): halves HBM traffic and
keeps rel err ~2e-3 against the 2e-2 gate (validated in fp64 emulation).

Per-core dataflow (unit = 2048 tokens; 2 units/batch, 8 units/core):
  - x loaded in natural (c-partition, token-free) bf16 layout, 512 KiB DMAs,
    prefetched one unit ahead.
  - |x|^2 entirely on PE: DVE squares xn into fp16 and pair-sums the two
    128-channel halves (2x mode, one unit ahead); 4 ones-basis matmuls
    reduce over channels into a (4 group, 512 token) PSUM tile (fp32-exact
    accumulation); after evac, one rank-4 f32r matmul folds scale_k * |x|^2
    into psl2.  No cross-layout shuffle needed.
  - PE is_transpose matmuls (bf16 identity -> 1 cyc/row) build xT tiles in
    bf16 PSUM; ACT/DVE/Pool evacuate them to SBUF for mm2.
  - mm1: psl2 (128 = 4 groups x 32 codes, 512 tokens) accumulates
    A = -2*scale*cw (bf16) against streamed bf16 x, one 32-col group per
    512-token group.
  - One ACT exp over (128, 512) with per-partition fp32 bias scale_k*|c_k|^2
    writes e as bf16.
  - Softmax denominators: PE matmul (bf16 group-indicator) -> (4, 512);
    DVE reciprocal; PE matmul broadcasts reciprocals back to (128, 512);
    DVE multiply normalizes -> w (bf16).
  - PE transposes w into (token, code) tiles (bf16 PSUM); DVE 2x-evacuates;
    mm2 (w stationary, xT moving, both bf16) accumulates out (32, 258) per
    batch; wsum rides col 256 via a ones-column in xT.
  - Final: one DVE scalar_tensor_tensor: out = cw*(-wsum) + wx; DMA out.
  - Unit u's softmax chain (exp..mm2) is interleaved into unit u+1's
    emission so each cross-engine hop overlaps transpose/mm1 work.
"""

import numpy as np
from contextlib import ExitStack

import ml_dtypes
import concourse.bass as bass
import concourse.bacc as bacc
import concourse.mybir as mybir
import concourse.tile as tile
from concourse.bass_utils import run_bass_kernel_spmd

F32 = mybir.dt.float32
F32R = mybir.dt.float32r
BF16 = mybir.dt.bfloat16
FP16 = mybir.dt.float16
ALU = mybir.AluOpType
ACTF = mybir.ActivationFunctionType

N_CORES = 8
B, C, K = 32, 256, 32
HW = 64 * 64            # 4096 tokens per batch
BL = B // N_CORES       # batches per core
UNIT = 2048             # tokens per unit
UNITS = BL * HW // UNIT  # 8 units per core
NCHUNK = 16             # 128-token chunks per unit
XTW = 258               # xT cols per chunk: 256 data + ones + pad


def build_module(bl=BL, debug=False):
    nc = bacc.Bacc(None)
    units = bl * HW // UNIT
    if debug:
        dbg_xT = nc.dram_tensor("dbg_xT", (128, NCHUNK * XTW), BF16, kind="ExternalOutput")
        dbg_q4 = nc.dram_tensor("dbg_q4", (4, 512), F32, kind="ExternalOutput")
        dbg_e = nc.dram_tensor("dbg_e", (128, 512), BF16, kind="ExternalOutput")
        dbg_wt = nc.dram_tensor("dbg_wt", (128, 512), BF16, kind="ExternalOutput")
        dbg_wtT = nc.dram_tensor("dbg_wtT", (128, 512), BF16, kind="ExternalOutput")

    x_d = nc.dram_tensor("x", (bl, 2, 128, HW), BF16, kind="ExternalInput")
    a_d = nc.dram_tensor("A", (128, 8, 128), BF16, kind="ExternalInput")
    onb_d = nc.dram_tensor("ONB", (128, 4, 4), FP16, kind="ExternalInput")
    scbd_d = nc.dram_tensor("SCBD", (4, 128), F32R, kind="ExternalInput")
    bias_d = nc.dram_tensor("BIASB", (128, 1), F32, kind="ExternalInput")
    gs_d = nc.dram_tensor("GS", (128, 4), BF16, kind="ExternalInput")
    gb_d = nc.dram_tensor("GB", (4, 128), BF16, kind="ExternalInput")
    cw_d = nc.dram_tensor("CWD", (32, 256), F32, kind="ExternalInput")
    onz_d = nc.dram_tensor("ONZ", (128, 32), BF16, kind="ExternalInput")
    idt_d = nc.dram_tensor("IDT", (128, 128), BF16, kind="ExternalInput")
    out_d = nc.dram_tensor("out", (bl, 32, 256), F32, kind="ExternalOutput")

    with tile.TileContext(nc) as tc, ExitStack() as ctx:
        sb = ctx.enter_context(tc.tile_pool(name="sb", bufs=2))
        sbx = ctx.enter_context(tc.tile_pool(name="sbx", bufs=3))
        cp = ctx.enter_context(tc.tile_pool(name="consts", bufs=1))
        ps_xt = ctx.enter_context(tc.tile_pool(name="ps_xt", bufs=2, space="PSUM"))
        ps_big = ctx.enter_context(tc.tile_pool(name="ps_big", bufs=2, space="PSUM"))
        ps_d = ctx.enter_context(tc.tile_pool(name="ps_d", bufs=1, space="PSUM"))
        ps_q = ctx.enter_context(tc.tile_pool(name="ps_q", bufs=1, space="PSUM"))
        ps_wtt = ctx.enter_context(tc.tile_pool(name="ps_wtt", bufs=1, space="PSUM"))
        ps_wx = ctx.enter_context(tc.tile_pool(name="ps_wx", bufs=1, space="PSUM"))

        def c(shape, dram, tag, dt):
            t = cp.tile(shape, dt, tag=tag)
            nc.sync.dma_start(t[:], dram[:])
            return t

        # the two tiny consts the first transposes/evacs need, then the
        # unit-0 x loads (they gate the pipeline), then the other consts.
        idt_s = c([128, 128], idt_d, "idt", BF16)
        onz_s = c([128, 32], onz_d, "onz", BF16)
        xn0 = sbx.tile([128, 2, UNIT], BF16, tag="xn")
        nc.sync.dma_start(xn0[:, 0], x_d[0, 0, :, 0:UNIT])
        nc.sync.dma_start(xn0[:, 1], x_d[0, 1, :, 0:UNIT])

        scbd_s = c([4, 128], scbd_d, "scbd", F32R)
        bias_s = c([128, 1], bias_d, "bias", F32)
        gs_s = c([128, 4], gs_d, "gs", BF16)
        gb_s = c([4, 128], gb_d, "gb", BF16)
        cw_s = c([32, 256], cw_d, "cw", F32)
        onb_s = cp.tile([128, 4, 4], FP16, tag="onb")
        nc.sync.dma_start(onb_s[:], onb_d[:])
        a_s = cp.tile([128, 8, 128], BF16, tag="a")
        nc.sync.dma_start(a_s[:], a_d[:])

        pwx = {}
        pending_out = []

        def load_xn(u, xn=None, first=False):
            """Load x natural, fill xT's ones columns, and square xn for the
            PE |x|^2 reduction.  All run one unit ahead of stage(u)."""
            b_, uu = u // 2, u % 2
            t0 = uu * UNIT
            if xn is None:
                xn = sbx.tile([128, 2, UNIT], BF16, tag="xn")
                nc.sync.dma_start(xn[:, 0], x_d[b_, 0, :, t0:t0 + UNIT])
                nc.sync.dma_start(xn[:, 1], x_d[b_, 1, :, t0:t0 + UNIT])
            xT = sbx.tile([128, NCHUNK * XTW], BF16, tag="xT")
            xTv = xT[:].rearrange("p (j c) -> p j c", c=XTW)
            # col 256 = ones (mm2 col 256 accumulates wsum), col 257 = pad.
            nc.gpsimd.tensor_copy(
                xTv[:, :, 256:258],
                onz_s[:].rearrange("p (j c) -> p j c", c=2))
            # xqs[c, n] = xn0[c,n]^2 + xn1[c,n]^2 (fp16, DVE 2x): halves
            # the PE channel-reduction matmuls for |x|^2.  For the first
            # unit skip the pair-sum: 8 q-matmuls on the raw squares start
            # the pipeline ~3 us earlier (xq0 is ready right after the
            # first DMA half lands).
            xq0 = sb.tile([128, UNIT], FP16, tag="xq0")
            nc.vector.tensor_tensor(xq0[:], xn[:, 0], xn[:, 0], ALU.mult)
            xq1 = sb.tile([128, UNIT], FP16, tag="xq1")
            nc.vector.tensor_tensor(xq1[:], xn[:, 1], xn[:, 1], ALU.mult)
            if first:
                return xn, xT, (xq0, xq1)
            xq = sbx.tile([128, UNIT], FP16, tag="xq")
            nc.vector.tensor_tensor(xq[:], xq0[:], xq1[:], ALU.add)
            return xn, xT, xq

        def stage(u, prev, xn, xT, xq):
            """Emit A(u) interleaved with B(prev)."""
            b_, uu = u // 2, u % 2

            xTv = xT[:].rearrange("p (j c) -> p j c", c=XTW)
            st = dict(xT=xT, b=b_, uu=uu, u=u)
            psl2 = ps_big.tile([128, 512], F32, tag="big")
            st["psl2"] = psl2

            def mm1_part(i):
                g, cc = divmod(i, 2)
                nc.tensor.matmul(
                    psl2[:, :],
                    a_s[:, cc * 4 + g, :],
                    xn[:, cc, g * 512:(g + 1) * 512],
                    start=(i == 0), stop=False, skip_group_check=True,
                )

            def q_part(q4, g):
                # q4[g, n'] += sum_c xqs[c, 512g + n']  (exact fp32)
                if isinstance(xq, tuple):   # first unit: raw per-cc squares
                    for cc in (0, 1):
                        nc.tensor.matmul(
                            q4[:, :],
                            onb_s[:, g, :],
                            xq[cc][:, g * 512:(g + 1) * 512],
                            start=(g == 0 and cc == 0),
                            stop=(g == 3 and cc == 1), skip_group_check=True,
                        )
                else:
                    nc.tensor.matmul(
                        q4[:, :],
                        onb_s[:, g, :],
                        xq[:, g * 512:(g + 1) * 512],
                        start=(g == 0), stop=(g == 3), skip_group_check=True,
                    )

            def tgroup(j2):
                # PE transposes for both cc halves of 2 chunks
                xtp = ps_xt.tile([128, 512], BF16, tag="xt")
                for h in (0, 1):
                    j = 2 * j2 + h
                    for cc in (0, 1):
                        nc.tensor.transpose(
                            xtp[:, h * 256 + cc * 128:h * 256 + cc * 128 + 128],
                            xn[:, cc, j * 128:j * 128 + 128],
                            idt_s[:],
                        )
                # evacuate both chunks in one strided op
                dst = xTv[:, 2 * j2:2 * j2 + 2, 0:256]
                src = xtp[:].rearrange("p (h c) -> p h c", c=256)
                # unit 0: DVE is busy squaring x for the first |x|^2 pass,
                # so its evac tiles would stall the transpose rotation.
                if j2 in (0, 2, 3, 5, 6, 7) or p is None:
                    nc.scalar.copy(dst, src)
                else:
                    nc.vector.tensor_copy(dst, src)

            p = prev  # may be None (first unit)

            # emit deferred batch-output stores: by now the STT that feeds
            # them has drained, so the DMA doesn't block the ACT sequencer.
            while pending_out:
                ob, outs = pending_out.pop(0)
                nc.scalar.dma_start(out_d[ob], outs[:])

            # |x|^2 channel-reduction: 8 accumulating matmuls, exact fp32.
            # For unit 0 they are emitted late (xq(0) is still being computed
            # when PE starts; transposes/mm1 only need xn).
            q4 = ps_q.tile([4, 512], F32, tag="q")
            if p is not None:
                for i in range(4):
                    q_part(q4, i)
            tgroup(0)
            if p is not None:
                e = sb.tile([128, 512], BF16, tag="e")
                nc.scalar.activation(e[:], p["psl2"][:], ACTF.Exp, bias=bias_s[:])
            tgroup(1)
            mm1_part(0)
            mm1_part(1)
            if p is not None:
                ps4 = ps_d.tile([4, 512], F32, tag="d")
                nc.tensor.matmul(ps4[:], gs_s[:], e[:])
            tgroup(2)
            mm1_part(2)
            if p is not None:
                r4 = sb.tile([4, 512], BF16, tag="r4")
                with nc.allow_low_precision(reason="1/d in bf16: per-token scale, cancels in out"):
                    nc.vector.reciprocal(r4[:], ps4[:])
            tgroup(3)
            mm1_part(3)
            if p is not None:
                pR = ps_big.tile([128, 512], F32, tag="big")
                nc.tensor.matmul(pR[:], gb_s[:], r4[:])
            tgroup(4)
            mm1_part(4)
            if p is not None:
                wt = sb.tile([128, 512], BF16, tag="wt")
                nc.vector.tensor_tensor(wt[:], e[:], pR[:], ALU.mult)
            tgroup(5)
            mm1_part(5)
            if p is None:
                for i in range(4):
                    q_part(q4, i)
            # evacuate |x|^2 row-block for the fold matmul
            q4s = sb.tile([4, 512], F32R, tag="q4s")
            nc.scalar.copy(q4s[:], q4[:].bitcast(F32R))
            tgroup(6)
            mm1_part(6)
            if p is not None:
                if debug and p["u"] == 0:
                    nc.scalar.dma_start(dbg_xT[:], p["xT"][:])
                    nc.scalar.dma_start(dbg_e[:], e[:])
                    nc.scalar.dma_start(dbg_wt[:], wt[:])
                pwtT = ps_wtt.tile([128, 512], BF16, tag="wtt")
                for sl in range(4):
                    # transpose of the (128, 128) slice: column-block g of
                    # the result is wT for token-chunk j = 4*g + sl.
                    nc.tensor.transpose(
                        pwtT[:, 128 * sl:128 * sl + 128],
                        wt[:, 128 * sl:128 * sl + 128],
                        idt_s[:],
                    )
            tgroup(7)
            mm1_part(7)
            if p is not None:
                wtTs = sb.tile([128, 512], BF16, tag="wtTs")
                nc.vector.tensor_copy(wtTs[:], pwtT[:])
                if debug and p["u"] == 0:
                    nc.scalar.dma_start(dbg_wtT[:], wtTs[:])
            # fold scale_k * |x|^2 into psl2 and close the accumulation
            if debug and u == 0:
                nc.scalar.dma_start(dbg_q4[:], q4s[:].bitcast(F32))
            nc.tensor.matmul(
                psl2[:, :], scbd_s[:], q4s[:],
                start=False, stop=True, skip_group_check=True,
            )
            if p is not None:
                emit_mm2(p, wtTs)
            # tail: prefetch + xbar + squares for u+1
            if u + 1 < units:
                nxt = load_xn(u + 1)
            else:
                nxt = (None, None, None)
            return st, nxt

        def emit_mm2(p, wtTs):
            b_, uu, xT = p["b"], p["uu"], p["xT"]
            if uu == 0:
                pwx[b_] = ps_wx.tile([32, XTW], F32, tag="wx", name="pwx")
            for j in range(NCHUNK):
                nc.tensor.matmul(
                    pwx[b_][:, 0:XTW],
                    wtTs[:, 128 * (j % 4) + 32 * (j // 4):128 * (j % 4) + 32 * (j // 4) + 32],
                    xT[:, XTW * j:XTW * j + XTW],
                    start=(uu == 0 and j == 0), stop=(uu == 1 and j == NCHUNK - 1),
                    skip_group_check=True,
                )
            if uu == 1:
                outs = sb.tile([32, 256], F32, tag="outs")
                nc.vector.scalar_tensor_tensor(
                    out=outs[:], in0=cw_s[:], scalar=pwx[b_][:, 256:257],
                    in1=pwx[b_][:, 0:256], op0=ALU.mult, op1=ALU.add,
                )
                pending_out.append((b_, outs))
                del pwx[b_]

        def stage_last(u, p, xn, xT, xq):
            """Final unit: close its logits early (q+mm1+fold front-loaded)
            and thread its softmax chain through the remaining transposes so
            the post-loop drain is short."""
            b_, uu = u // 2, u % 2
            xTv = xT[:].rearrange("p (j c) -> p j c", c=XTW)
            st = dict(xT=xT, b=b_, uu=uu, u=u)
            psl2 = ps_big.tile([128, 512], F32, tag="big")
            st["psl2"] = psl2

            def mm1_part(i):
                g, cc = divmod(i, 2)
                nc.tensor.matmul(
                    psl2[:, :], a_s[:, cc * 4 + g, :],
                    xn[:, cc, g * 512:(g + 1) * 512],
                    start=(i == 0), stop=False, skip_group_check=True,
                )

            def tgroup(j2):
                xtp = ps_xt.tile([128, 512], BF16, tag="xt")
                for h in (0, 1):
                    j = 2 * j2 + h
                    for cc in (0, 1):
                        nc.tensor.transpose(
                            xtp[:, h * 256 + cc * 128:h * 256 + cc * 128 + 128],
                            xn[:, cc, j * 128:j * 128 + 128], idt_s[:],
                        )
                dst = xTv[:, 2 * j2:2 * j2 + 2, 0:256]
                srcv = xtp[:].rearrange("p (h c) -> p h c", c=256)
                if j2 in (0, 2, 3, 5, 6, 7):
                    nc.scalar.copy(dst, srcv)
                else:
                    nc.vector.tensor_copy(dst, srcv)

            while pending_out:
                ob, outs = pending_out.pop(0)
                nc.scalar.dma_start(out_d[ob], outs[:])

            e_p = sb.tile([128, 512], BF16, tag="e")
            nc.scalar.activation(e_p[:], p["psl2"][:], ACTF.Exp, bias=bias_s[:])
            q4 = ps_q.tile([4, 512], F32, tag="q")
            for g in range(4):
                nc.tensor.matmul(
                    q4[:, :], onb_s[:, g, :], xq[:, g * 512:(g + 1) * 512],
                    start=(g == 0), stop=(g == 3), skip_group_check=True,
                )
            for i in range(4):
                mm1_part(i)
            ps4_p = ps_d.tile([4, 512], F32, tag="d")
            nc.tensor.matmul(ps4_p[:], gs_s[:], e_p[:])
            for i in range(4, 8):
                mm1_part(i)
            r4_p = sb.tile([4, 512], BF16, tag="r4")
            with nc.allow_low_precision(reason="1/d in bf16: per-token scale, cancels in out"):
                nc.vector.reciprocal(r4_p[:], ps4_p[:])
            q4s = sb.tile([4, 512], F32R, tag="q4s")
            nc.scalar.copy(q4s[:], q4[:].bitcast(F32R))
            nc.tensor.matmul(
                psl2[:, :], scbd_s[:], q4s[:],
                start=False, stop=True, skip_group_check=True,
            )
            e_u = sb.tile([128, 512], BF16, tag="e")
            nc.scalar.activation(e_u[:], psl2[:], ACTF.Exp, bias=bias_s[:])
            tgroup(0)
            tgroup(1)
            pR_p = ps_big.tile([128, 512], F32, tag="big")
            nc.tensor.matmul(pR_p[:], gb_s[:], r4_p[:])
            tgroup(2)
            wt_p = sb.tile([128, 512], BF16, tag="wt")
            nc.vector.tensor_tensor(wt_p[:], e_p[:], pR_p[:], ALU.mult)
            ps4_u = ps_d.tile([4, 512], F32, tag="d")
            nc.tensor.matmul(ps4_u[:], gs_s[:], e_u[:])
            tgroup(3)
            r4_u = sb.tile([4, 512], BF16, tag="r4")
            with nc.allow_low_precision(reason="1/d in bf16: per-token scale, cancels in out"):
                nc.vector.reciprocal(r4_u[:], ps4_u[:])
            pwtT_p = ps_wtt.tile([128, 512], BF16, tag="wtt")
            for sl in range(4):
                nc.tensor.transpose(
                    pwtT_p[:, 128 * sl:128 * sl + 128],
                    wt_p[:, 128 * sl:128 * sl + 128], idt_s[:],
                )
            tgroup(4)
            pR_u = ps_big.tile([128, 512], F32, tag="big")
            nc.tensor.matmul(pR_u[:], gb_s[:], r4_u[:])
            wtTs_p = sb.tile([128, 512], BF16, tag="wtTs")
            nc.vector.tensor_copy(wtTs_p[:], pwtT_p[:])
            tgroup(5)
            wt_u = sb.tile([128, 512], BF16, tag="wt")
            nc.vector.tensor_tensor(wt_u[:], e_u[:], pR_u[:], ALU.mult)
            emit_mm2(p, wtTs_p)
            tgroup(6)
            tgroup(7)
            return st, wt_u

        cur = load_xn(0, xn=xn0, first=True)
        prev = None
        for u in range(units - 1):
            prev, cur = stage(u, prev, *cur)
        prev, wt_last = stage_last(units - 1, prev, *cur)
        # epilogue: only the tail of the last unit's chain remains
        pwtT = ps_wtt.tile([128, 512], BF16, tag="wtt")
        for sl in range(4):
            nc.tensor.transpose(
                pwtT[:, 128 * sl:128 * sl + 128],
                wt_last[:, 128 * sl:128 * sl + 128],
                idt_s[:],
            )
        wtTs = sb.tile([128, 512], BF16, tag="wtTs")
        nc.vector.tensor_copy(wtTs[:], pwtT[:])
        emit_mm2(prev, wtTs)
        while pending_out:
            ob, outs = pending_out.pop(0)
            nc.scalar.dma_start(out_d[ob], outs[:])

    nc.finalize()
    return nc


def host_constants(codewords, scale):
    cw = np.asarray(codewords, dtype=np.float32)
    sc = np.asarray(scale, dtype=np.float32)
    c_sq = (cw.astype(np.float64) ** 2).sum(-1).astype(np.float32)

    A = np.zeros((2, 4, 128, 128), np.float32)
    for cc in range(2):
        blk = (-2.0 * sc[None, :]) * cw[:, cc * 128:(cc + 1) * 128].T
        for g in range(4):
            A[cc, g, :, 32 * g:32 * g + 32] = blk

    ONB = np.zeros((128, 4, 4), np.float32)
    for g in range(4):
        ONB[:, g, g] = 1.0

    SCBD = np.zeros((4, 128), np.float32)
    BIASB = np.zeros((128, 1), np.float32)
    GS = np.zeros((128, 4), np.float32)
    GB = np.zeros((4, 128), np.float32)
    for g in range(4):
        SCBD[g, 32 * g:32 * g + 32] = sc
        BIASB[32 * g:32 * g + 32, 0] = sc * c_sq
        GS[32 * g:32 * g + 32, g] = 1.0
        GB[g, 32 * g:32 * g + 32] = 1.0

    bf = ml_dtypes.bfloat16
    return {
        "A": np.ascontiguousarray(A.transpose(2, 0, 1, 3).reshape(128, 8, 128)).astype(bf),
        "ONB": ONB.astype(np.float16), "SCBD": SCBD,
        "BIASB": BIASB, "GS": GS.astype(bf), "GB": GB.astype(bf),
        "CWD": np.ascontiguousarray(-cw),
        "ONZ": np.tile(np.array([1.0, 0.0], bf), (128, 16)),
        "IDT": np.eye(128, dtype=bf),
    }


def make_in_maps(x, codewords, scale):
    consts = host_constants(codewords, scale)
    xb = np.asarray(x).astype(ml_dtypes.bfloat16)
    xs = xb.reshape(B, 2, 128, HW)
    in_maps = []
    for i in range(N_CORES):
        m = dict(consts)
        m["x"] = np.ascontiguousarray(xs[BL * i:BL * (i + 1)])
        in_maps.append(m)
    return in_maps


_CACHE = {}


def kernel(x, codewords, scale):
    if "nc" not in _CACHE:
        _CACHE["nc"] = build_module()
    nc = _CACHE["nc"]
    in_maps = make_in_maps(x, codewords, scale)
    res = run_bass_kernel_spmd(nc, in_maps, list(range(N_CORES)))
    out = np.concatenate([r["out"] for r in res.results], axis=0)
    return out.astype(np.float32)


# revision 56
# speedup vs baseline: 70804.8013x; 1.0152x over previous
"""Trainium2 Bass kernel for nn_Encoding (vq_codebook).

Math (per batch b):
    xf = x[b].reshape(C, N).T                      # (N tokens, C)
    sl2[n,k] = scale[k] * (|xf_n|^2 - 2 xf_n.c_k + |c_k|^2)
    w = softmax_k(sl2)                             # max-subtract skipped: sl2 in (-600, -0.18]
    out[b] = w.T @ xf - (sum_n w)[:,None] * codewords

Sharding: data-parallel over batch B=32 -> 4 batches per core on 8 cores.
x is shipped to the device as bf16 (host cast): halves HBM traffic and
keeps rel err ~2e-3 against the 2e-2 gate (validated in fp64 emulation).

Per-core dataflow (unit = 2048 tokens; 2 units/batch, 8 units/core):
  - x loaded in natural (c-partition, token-free) bf16 layout, 512 KiB DMAs,
    prefetched one unit ahead.
  - |x|^2 entirely on PE: DVE squares xn into fp16 and pair-sums the two
    128-channel halves (2x mode, one unit ahead); 4 ones-basis matmuls
    reduce over channels into a (4 group, 512 token) PSUM tile (fp32-exact
    accumulation); after evac, one rank-4 f32r matmul folds scale_k * |x|^2
    into psl2.  No cross-layout shuffle needed.
  - PE is_transpose matmuls (bf16 identity -> 1 cyc/row) build xT tiles in
    bf16 PSUM; ACT/DVE/Pool evacuate them to SBUF for mm2.
  - mm1: psl2 (128 = 4 groups x 32 codes, 512 tokens) accumulates
    A = -2*scale*cw (bf16) against streamed bf16 x, one 32-col group per
    512-token group.
  - One ACT exp over (128, 512) with per-partition fp32 bias scale_k*|c_k|^2
    writes e as bf16.
  - Softmax denominators: PE matmul (bf16 group-indicator) -> (4, 512);
    DVE reciprocal; PE matmul broadcasts reciprocals back to (128, 512);
    DVE multiply normalizes -> w (bf16).
  - PE transposes w into (token, code) tiles (bf16 PSUM); DVE 2x-evacuates;
    mm2 (w stationary, xT moving, both bf16) accumulates out (32, 258) per
    batch; wsum rides col 256 via a ones-column in xT.
  - Final: one DVE scalar_tensor_tensor: out = cw*(-wsum) + wx; DMA out.
  - Unit u's softmax chain (exp..mm2) is interleaved into unit u+1's
    emission so each cross-engine hop overlaps transpose/mm1 work.
"""

import numpy as np
from contextlib import ExitStack

import ml_dtypes
import concourse.bass as bass
import concourse.bacc as bacc
import concourse.mybir as mybir
import concourse.tile as tile
from concourse.bass_utils import run_bass_kernel_spmd

F32 = mybir.dt.float32
F32R = mybir.dt.float32r
BF16 = mybir.dt.bfloat16
FP16 = mybir.dt.float16
ALU = mybir.AluOpType
ACTF = mybir.ActivationFunctionType

N_CORES = 8
B, C, K = 32, 256, 32
HW = 64 * 64            # 4096 tokens per batch
BL = B // N_CORES       # batches per core
UNIT = 2048             # tokens per unit
UNITS = BL * HW // UNIT  # 8 units per core
NCHUNK = 16             # 128-token chunks per unit
XTW = 258               # xT cols per chunk: 256 data + ones + pad


def build_module(bl=BL, debug=False):
    nc = bacc.Bacc(None)
    units = bl * HW // UNIT
    if debug:
        dbg_xT = nc.dram_tensor("dbg_xT", (128, NCHUNK * XTW), BF16, kind="ExternalOutput")
        dbg_q4 = nc.dram_tensor("dbg_q4", (4, 512), F32, kind="ExternalOutput")
        dbg_e = nc.dram_tensor("dbg_e", (128, 512), BF16, kind="ExternalOutput")
        dbg_wt = nc.dram_tensor("dbg_wt", (128, 512), BF16, kind="ExternalOutput")
        dbg_wtT = nc.dram_tensor("dbg_wtT", (128, 512), BF16, kind="ExternalOutput")

    x_d = nc.dram_tensor("x", (bl, 2, 128, HW), BF16, kind="ExternalInput")
    a_d = nc.dram_tensor("A", (128, 8, 128), BF16, kind="ExternalInput")
    onb_d = nc.dram_tensor("ONB", (128, 4, 128), FP16, kind="ExternalInput")
    scl_d = nc.dram_tensor("SCL", (128, 1), F32, kind="ExternalInput")
    bias_d = nc.dram_tensor("BIASB", (128, 1), F32, kind="ExternalInput")
    gs_d = nc.dram_tensor("GS", (128, 4), BF16, kind="ExternalInput")
    gb_d = nc.dram_tensor("GB", (4, 128), BF16, kind="ExternalInput")
    cw_d = nc.dram_tensor("CWD", (32, 256), F32, kind="ExternalInput")
    onz_d = nc.dram_tensor("ONZ", (128, 32), BF16, kind="ExternalInput")
    idt_d = nc.dram_tensor("IDT", (128, 128), BF16, kind="ExternalInput")
    out_d = nc.dram_tensor("out", (bl, 32, 256), F32, kind="ExternalOutput")

    with tile.TileContext(nc) as tc, ExitStack() as ctx:
        sb = ctx.enter_context(tc.tile_pool(name="sb", bufs=2))
        sbx = ctx.enter_context(tc.tile_pool(name="sbx", bufs=3))
        cp = ctx.enter_context(tc.tile_pool(name="consts", bufs=1))
        ps_xt = ctx.enter_context(tc.tile_pool(name="ps_xt", bufs=3, space="PSUM"))
        ps_big = ctx.enter_context(tc.tile_pool(name="ps_big", bufs=2, space="PSUM"))
        ps_d = ctx.enter_context(tc.tile_pool(name="ps_d", bufs=1, space="PSUM"))
        ps_wtt = ctx.enter_context(tc.tile_pool(name="ps_wtt", bufs=1, space="PSUM"))
        ps_wx = ctx.enter_context(tc.tile_pool(name="ps_wx", bufs=1, space="PSUM"))

        def c(shape, dram, tag, dt):
            t = cp.tile(shape, dt, tag=tag)
            nc.sync.dma_start(t[:], dram[:])
            return t

        # the two tiny consts the first transposes/evacs need, then the
        # unit-0 x loads (they gate the pipeline), then the other consts.
        idt_s = c([128, 128], idt_d, "idt", BF16)
        onz_s = c([128, 32], onz_d, "onz", BF16)
        xn0 = sbx.tile([128, 2, UNIT], BF16, tag="xn")
        nc.sync.dma_start(xn0[:, 0], x_d[0, 0, :, 0:UNIT])
        nc.sync.dma_start(xn0[:, 1], x_d[0, 1, :, 0:UNIT])

        scl_s = c([128, 1], scl_d, "scl", F32)
        bias_s = c([128, 1], bias_d, "bias", F32)
        gs_s = c([128, 4], gs_d, "gs", BF16)
        gb_s = c([4, 128], gb_d, "gb", BF16)
        cw_s = c([32, 256], cw_d, "cw", F32)
        onb_s = cp.tile([128, 4, 128], FP16, tag="onb")
        nc.sync.dma_start(onb_s[:], onb_d[:])
        a_s = cp.tile([128, 8, 128], BF16, tag="a")
        nc.sync.dma_start(a_s[:], a_d[:])

        pwx = {}
        pending_out = []

        def load_xn(u, xn=None, first=False):
            """Load x natural, fill xT's ones columns, and square xn for the
            PE |x|^2 reduction.  All run one unit ahead of stage(u)."""
            b_, uu = u // 2, u % 2
            t0 = uu * UNIT
            if xn is None:
                xn = sbx.tile([128, 2, UNIT], BF16, tag="xn")
                nc.sync.dma_start(xn[:, 0], x_d[b_, 0, :, t0:t0 + UNIT])
                nc.sync.dma_start(xn[:, 1], x_d[b_, 1, :, t0:t0 + UNIT])
            xT = sbx.tile([128, NCHUNK * XTW], BF16, tag="xT")
            xTv = xT[:].rearrange("p (j c) -> p j c", c=XTW)
            # col 256 = ones (mm2 col 256 accumulates wsum), col 257 = pad.
            nc.gpsimd.tensor_copy(
                xTv[:, :, 256:258],
                onz_s[:].rearrange("p (j c) -> p j c", c=2))
            # xqs[c, n] = xn0[c,n]^2 + xn1[c,n]^2 (fp16, DVE 2x): halves
            # the PE channel-reduction matmuls for |x|^2.  For the first
            # unit skip the pair-sum: 8 q-matmuls on the raw squares start
            # the pipeline ~3 us earlier (xq0 is ready right after the
            # first DMA half lands).
            xq0 = sb.tile([128, UNIT], FP16, tag="xq0")
            nc.vector.tensor_tensor(xq0[:], xn[:, 0], xn[:, 0], ALU.mult)
            xq1 = sb.tile([128, UNIT], FP16, tag="xq1")
            nc.vector.tensor_tensor(xq1[:], xn[:, 1], xn[:, 1], ALU.mult)
            if first:
                return xn, xT, (xq0, xq1)
            xq = sbx.tile([128, UNIT], FP16, tag="xq")
            nc.vector.tensor_tensor(xq[:], xq0[:], xq1[:], ALU.add)
            return xn, xT, xq

        def stage(u, prev, xn, xT, xq):
            """Emit A(u) interleaved with B(prev)."""
            b_, uu = u // 2, u % 2

            xTv = xT[:].rearrange("p (j c) -> p j c", c=XTW)
            st = dict(xT=xT, b=b_, uu=uu, u=u)
            psl2 = ps_big.tile([128, 512], F32, tag="big")
            st["psl2"] = psl2

            def mm1_part(i, start=False, stop=False):
                g, cc = divmod(i, 2)
                nc.tensor.matmul(
                    psl2[:, :],
                    a_s[:, cc * 4 + g, :],
                    xn[:, cc, g * 512:(g + 1) * 512],
                    start=start, stop=stop, skip_group_check=True,
                )

            def q_part(g, start=False, stop=False):
                # psl2[32g+k, n'] += sum_c xqs[c, 512g + n']  (exact fp32,
                # unscaled; exp applies scale_k per partition)
                if isinstance(xq, tuple):   # first unit: raw per-cc squares
                    for cc in (0, 1):
                        nc.tensor.matmul(
                            psl2[:, :],
                            onb_s[:, g, :],
                            xq[cc][:, g * 512:(g + 1) * 512],
                            start=(start and cc == 0),
                            stop=(stop and cc == 1), skip_group_check=True,
                        )
                else:
                    nc.tensor.matmul(
                        psl2[:, :],
                        onb_s[:, g, :],
                        xq[:, g * 512:(g + 1) * 512],
                        start=start, stop=stop, skip_group_check=True,
                    )

            def tgroup(j2):
                # PE transposes for both cc halves of 2 chunks
                xtp = ps_xt.tile([128, 512], BF16, tag="xt")
                for h in (0, 1):
                    j = 2 * j2 + h
                    for cc in (0, 1):
                        nc.tensor.transpose(
                            xtp[:, h * 256 + cc * 128:h * 256 + cc * 128 + 128],
                            xn[:, cc, j * 128:j * 128 + 128],
                            idt_s[:],
                        )
                # evacuate both chunks in one strided op
                dst = xTv[:, 2 * j2:2 * j2 + 2, 0:256]
                src = xtp[:].rearrange("p (h c) -> p h c", c=256)
                # unit 0: DVE is busy squaring x for the first |x|^2 pass,
                # so its evac tiles would stall the transpose rotation.
                if j2 in (0, 2, 3, 5, 6, 7) or p is None:
                    nc.scalar.copy(dst, src)
                else:
                    nc.vector.tensor_copy(dst, src)

            p = prev  # may be None (first unit)

            # emit deferred batch-output stores: by now the STT that feeds
            # them has drained, so the DMA doesn't block the ACT sequencer.
            while pending_out:
                ob, outs = pending_out.pop(0)
                nc.scalar.dma_start(out_d[ob], outs[:])

            # |x|^2 channel-reduction matmuls accumulate straight into
            # psl2.  For unit 0 they are emitted late (xq(0) is still being
            # computed when PE starts; transposes/mm1 only need xn).
            if p is not None:
                for i in range(4):
                    q_part(i, start=(i == 0))
            tgroup(0)
            if p is not None:
                e = sb.tile([128, 512], BF16, tag="e")
                nc.scalar.activation(e[:], p["psl2"][:], ACTF.Exp,
                                     bias=bias_s[:], scale=scl_s[:])
            tgroup(1)
            mm1_part(0, start=(p is None))
            mm1_part(1)
            if p is not None:
                ps4 = ps_d.tile([4, 512], F32, tag="d")
                nc.tensor.matmul(ps4[:], gs_s[:], e[:])
            tgroup(2)
            mm1_part(2)
            if p is not None:
                r4 = sb.tile([4, 512], BF16, tag="r4")
                with nc.allow_low_precision(reason="1/d in bf16: per-token scale, cancels in out"):
                    nc.vector.reciprocal(r4[:], ps4[:])
            tgroup(3)
            mm1_part(3)
            if p is not None:
                pR = ps_big.tile([128, 512], F32, tag="big")
                nc.tensor.matmul(pR[:], gb_s[:], r4[:])
            tgroup(4)
            mm1_part(4)
            if p is not None:
                wt = sb.tile([128, 512], BF16, tag="wt")
                nc.vector.tensor_tensor(wt[:], e[:], pR[:], ALU.mult)
            tgroup(5)
            mm1_part(5)
            tgroup(6)
            mm1_part(6)
            if p is not None:
                if debug and p["u"] == 0:
                    nc.scalar.dma_start(dbg_xT[:], p["xT"][:])
                    nc.scalar.dma_start(dbg_e[:], e[:])
                    nc.scalar.dma_start(dbg_wt[:], wt[:])
                pwtT = ps_wtt.tile([128, 512], BF16, tag="wtt")
                for sl in range(4):
                    # transpose of the (128, 128) slice: column-block g of
                    # the result is wT for token-chunk j = 4*g + sl.
                    nc.tensor.transpose(
                        pwtT[:, 128 * sl:128 * sl + 128],
                        wt[:, 128 * sl:128 * sl + 128],
                        idt_s[:],
                    )
            tgroup(7)
            mm1_part(7, stop=(p is not None))
            if p is not None:
                wtTs = sb.tile([128, 512], BF16, tag="wtTs")
                nc.vector.tensor_copy(wtTs[:], pwtT[:])
                if debug and p["u"] == 0:
                    nc.scalar.dma_start(dbg_wtT[:], wtTs[:])
            if p is None:
                # unit 0: late q matmuls close the accumulation
                for i in range(4):
                    q_part(i, stop=(i == 3))
            if p is not None:
                emit_mm2(p, wtTs)
            # tail: prefetch + xbar + squares for u+1
            if u + 1 < units:
                nxt = load_xn(u + 1)
            else:
                nxt = (None, None, None)
            return st, nxt

        def emit_mm2(p, wtTs):
            b_, uu, xT = p["b"], p["uu"], p["xT"]
            if uu == 0:
                pwx[b_] = ps_wx.tile([32, XTW], F32, tag="wx", name="pwx")
            for j in range(NCHUNK):
                nc.tensor.matmul(
                    pwx[b_][:, 0:XTW],
                    wtTs[:, 128 * (j % 4) + 32 * (j // 4):128 * (j % 4) + 32 * (j // 4) + 32],
                    xT[:, XTW * j:XTW * j + XTW],
                    start=(uu == 0 and j == 0), stop=(uu == 1 and j == NCHUNK - 1),
                    skip_group_check=True,
                )
            if uu == 1:
                outs = sb.tile([32, 256], F32, tag="outs")
                nc.vector.scalar_tensor_tensor(
                    out=outs[:], in0=cw_s[:], scalar=pwx[b_][:, 256:257],
                    in1=pwx[b_][:, 0:256], op0=ALU.mult, op1=ALU.add,
                )
                pending_out.append((b_, outs))
                del pwx[b_]

        def stage_last(u, p, xn, xT, xq):
            """Final unit: close its logits early (q+mm1+fold front-loaded)
            and thread its softmax chain through the remaining transposes so
            the post-loop drain is short."""
            b_, uu = u // 2, u % 2
            xTv = xT[:].rearrange("p (j c) -> p j c", c=XTW)
            st = dict(xT=xT, b=b_, uu=uu, u=u)
            psl2 = ps_big.tile([128, 512], F32, tag="big")
            st["psl2"] = psl2

            def mm1_part(i, stop=False):
                g, cc = divmod(i, 2)
                nc.tensor.matmul(
                    psl2[:, :], a_s[:, cc * 4 + g, :],
                    xn[:, cc, g * 512:(g + 1) * 512],
                    start=False, stop=stop, skip_group_check=True,
                )

            def tgroup(j2):
                xtp = ps_xt.tile([128, 512], BF16, tag="xt")
                for h in (0, 1):
                    j = 2 * j2 + h
                    for cc in (0, 1):
                        nc.tensor.transpose(
                            xtp[:, h * 256 + cc * 128:h * 256 + cc * 128 + 128],
                            xn[:, cc, j * 128:j * 128 + 128], idt_s[:],
                        )
                dst = xTv[:, 2 * j2:2 * j2 + 2, 0:256]
                srcv = xtp[:].rearrange("p (h c) -> p h c", c=256)
                if j2 in (0, 2, 3, 5, 6, 7):
                    nc.scalar.copy(dst, srcv)
                else:
                    nc.vector.tensor_copy(dst, srcv)

            while pending_out:
                ob, outs = pending_out.pop(0)
                nc.scalar.dma_start(out_d[ob], outs[:])

            e_p = sb.tile([128, 512], BF16, tag="e")
            nc.scalar.activation(e_p[:], p["psl2"][:], ACTF.Exp,
                                 bias=bias_s[:], scale=scl_s[:])
            for g in range(4):
                nc.tensor.matmul(
                    psl2[:, :], onb_s[:, g, :], xq[:, g * 512:(g + 1) * 512],
                    start=(g == 0), stop=False, skip_group_check=True,
                )
            for i in range(4):
                mm1_part(i)
            ps4_p = ps_d.tile([4, 512], F32, tag="d")
            nc.tensor.matmul(ps4_p[:], gs_s[:], e_p[:])
            for i in range(4, 8):
                mm1_part(i, stop=(i == 7))
            r4_p = sb.tile([4, 512], BF16, tag="r4")
            with nc.allow_low_precision(reason="1/d in bf16: per-token scale, cancels in out"):
                nc.vector.reciprocal(r4_p[:], ps4_p[:])
            e_u = sb.tile([128, 512], BF16, tag="e")
            nc.scalar.activation(e_u[:], psl2[:], ACTF.Exp,
                                 bias=bias_s[:], scale=scl_s[:])
            tgroup(0)
            tgroup(1)
            pR_p = ps_big.tile([128, 512], F32, tag="big")
            nc.tensor.matmul(pR_p[:], gb_s[:], r4_p[:])
            tgroup(2)
            wt_p = sb.tile([128, 512], BF16, tag="wt")
            nc.vector.tensor_tensor(wt_p[:], e_p[:], pR_p[:], ALU.mult)
            ps4_u = ps_d.tile([4, 512], F32, tag="d")
            nc.tensor.matmul(ps4_u[:], gs_s[:], e_u[:])
            tgroup(3)
            r4_u = sb.tile([4, 512], BF16, tag="r4")
            with nc.allow_low_precision(reason="1/d in bf16: per-token scale, cancels in out"):
                nc.vector.reciprocal(r4_u[:], ps4_u[:])
            pwtT_p = ps_wtt.tile([128, 512], BF16, tag="wtt")
            for sl in range(4):
                nc.tensor.transpose(
                    pwtT_p[:, 128 * sl:128 * sl + 128],
                    wt_p[:, 128 * sl:128 * sl + 128], idt_s[:],
                )
            tgroup(4)
            pR_u = ps_big.tile([128, 512], F32, tag="big")
            nc.tensor.matmul(pR_u[:], gb_s[:], r4_u[:])
            wtTs_p = sb.tile([128, 512], BF16, tag="wtTs")
            nc.vector.tensor_copy(wtTs_p[:], pwtT_p[:])
            tgroup(5)
            wt_u = sb.tile([128, 512], BF16, tag="wt")
            nc.vector.tensor_tensor(wt_u[:], e_u[:], pR_u[:], ALU.mult)
            emit_mm2(p, wtTs_p)
            tgroup(6)
            tgroup(7)
            return st, wt_u

        cur = load_xn(0, xn=xn0, first=True)
        prev = None
        for u in range(units - 1):
            prev, cur = stage(u, prev, *cur)
        prev, wt_last = stage_last(units - 1, prev, *cur)
        # epilogue: only the tail of the last unit's chain remains
        pwtT = ps_wtt.tile([128, 512], BF16, tag="wtt")
        for sl in range(4):
            nc.tensor.transpose(
                pwtT[:, 128 * sl:128 * sl + 128],
                wt_last[:, 128 * sl:128 * sl + 128],
                idt_s[:],
            )
        wtTs = sb.tile([128, 512], BF16, tag="wtTs")
        nc.vector.tensor_copy(wtTs[:], pwtT[:])
        emit_mm2(prev, wtTs)
        while pending_out:
            ob, outs = pending_out.pop(0)
            nc.scalar.dma_start(out_d[ob], outs[:])

    nc.finalize()
    return nc


def host_constants(codewords, scale):
    cw = np.asarray(codewords, dtype=np.float32)
    sc = np.asarray(scale, dtype=np.float32)
    c_sq = (cw.astype(np.float64) ** 2).sum(-1).astype(np.float32)

    A = np.zeros((2, 4, 128, 128), np.float32)
    for cc in range(2):
        blk = -2.0 * cw[:, cc * 128:(cc + 1) * 128].T
        for g in range(4):
            A[cc, g, :, 32 * g:32 * g + 32] = blk

    ONB = np.zeros((128, 4, 128), np.float32)
    SCL = np.zeros((128, 1), np.float32)
    BIASB = np.zeros((128, 1), np.float32)
    GS = np.zeros((128, 4), np.float32)
    GB = np.zeros((4, 128), np.float32)
    for g in range(4):
        ONB[:, g, 32 * g:32 * g + 32] = 1.0
        SCL[32 * g:32 * g + 32, 0] = sc
        BIASB[32 * g:32 * g + 32, 0] = sc * c_sq
        GS[32 * g:32 * g + 32, g] = 1.0
        GB[g, 32 * g:32 * g + 32] = 1.0

    bf = ml_dtypes.bfloat16
    return {
        "A": np.ascontiguousarray(A.transpose(2, 0, 1, 3).reshape(128, 8, 128)).astype(bf),
        "ONB": ONB.astype(np.float16), "SCL": SCL,
        "BIASB": BIASB, "GS": GS.astype(bf), "GB": GB.astype(bf),
        "CWD": np.ascontiguousarray(-cw),
        "ONZ": np.tile(np.array([1.0, 0.0], bf), (128, 16)),
        "IDT": np.eye(128, dtype=bf),
    }


def make_in_maps(x, codewords, scale):
    consts = host_constants(codewords, scale)
    xb = np.asarray(x).astype(ml_dtypes.bfloat16)
    xs = xb.reshape(B, 2, 128, HW)
    in_maps = []
    for i in range(N_CORES):
        m = dict(consts)
        m["x"] = np.ascontiguousarray(xs[BL * i:BL * (i + 1)])
        in_maps.append(m)
    return in_maps


_CACHE = {}


def kernel(x, codewords, scale):
    if "nc" not in _CACHE:
        _CACHE["nc"] = build_module()
    nc = _CACHE["nc"]
    in_maps = make_in_maps(x, codewords, scale)
    res = run_bass_kernel_spmd(nc, in_maps, list(range(N_CORES)))
    out = np.concatenate([r["out"] for r in res.results], axis=0)
    return out.astype(np.float32)


# revision 65
# speedup vs baseline: 71726.6157x; 1.0130x over previous
"""Trainium2 Bass kernel for nn_Encoding (vq_codebook).

Math (per batch b):
    xf = x[b].reshape(C, N).T                      # (N tokens, C)
    sl2[n,k] = scale[k] * (|xf_n|^2 - 2 xf_n.c_k + |c_k|^2)
    w = softmax_k(sl2)                             # max-subtract skipped: sl2 in (-600, -0.18]
    out[b] = w.T @ xf - (sum_n w)[:,None] * codewords

Sharding: data-parallel over batch B=32 -> 4 batches per core on 8 cores.
x is shipped to the device as bf16 (host cast): halves HBM traffic and
keeps rel err ~2e-3 against the 2e-2 gate (validated in fp64 emulation).

Per-core dataflow (unit = 2048 tokens; 2 units/batch, 8 units/core):
  - x loaded in natural (c-partition, token-free) bf16 layout, 512 KiB DMAs,
    prefetched one unit ahead.
  - |x|^2 entirely on PE: DVE squares xn into fp16 and pair-sums the two
    128-channel halves (2x mode, one unit ahead); 4 ones-basis matmuls
    reduce over channels into a (4 group, 512 token) PSUM tile (fp32-exact
    accumulation); after evac, one rank-4 f32r matmul folds scale_k * |x|^2
    into psl2.  No cross-layout shuffle needed.
  - PE is_transpose matmuls (bf16 identity -> 1 cyc/row) build xT tiles in
    bf16 PSUM; ACT/DVE/Pool evacuate them to SBUF for mm2.
  - mm1: psl2 (128 = 4 groups x 32 codes, 512 tokens) accumulates
    A = -2*scale*cw (bf16) against streamed bf16 x, one 32-col group per
    512-token group.
  - One ACT exp over (128, 512) with per-partition fp32 bias scale_k*|c_k|^2
    writes e as bf16.
  - Softmax denominators: PE matmul (bf16 group-indicator) -> (4, 512);
    DVE reciprocal; PE matmul broadcasts reciprocals back to (128, 512);
    DVE multiply normalizes -> w (bf16).
  - PE transposes w into (token, code) tiles (bf16 PSUM); DVE 2x-evacuates;
    mm2 (w stationary, xT moving, both bf16) accumulates out (32, 258) per
    batch; wsum rides col 256 via a ones-column in xT.
  - Final: one DVE scalar_tensor_tensor: out = cw*(-wsum) + wx; DMA out.
  - Unit u's softmax chain (exp..mm2) is interleaved into unit u+1's
    emission so each cross-engine hop overlaps transpose/mm1 work.
"""

import numpy as np
from contextlib import ExitStack

import ml_dtypes
import concourse.bass as bass
import concourse.bacc as bacc
import concourse.mybir as mybir
import concourse.tile as tile
from concourse.bass_utils import run_bass_kernel_spmd

F32 = mybir.dt.float32
F32R = mybir.dt.float32r
BF16 = mybir.dt.bfloat16
FP16 = mybir.dt.float16
ALU = mybir.AluOpType
ACTF = mybir.ActivationFunctionType

N_CORES = 8
B, C, K = 32, 256, 32
HW = 64 * 64            # 4096 tokens per batch
BL = B // N_CORES       # batches per core
UNIT = 2048             # tokens per unit
UNITS = BL * HW // UNIT  # 8 units per core
NCHUNK = 16             # 128-token chunks per unit
XTW = 258               # xT cols per chunk: 256 data + ones + pad


def build_module(bl=BL, debug=False):
    nc = bacc.Bacc(None)
    units = bl * HW // UNIT
    if debug:
        dbg_xT = nc.dram_tensor("dbg_xT", (128, NCHUNK * XTW), BF16, kind="ExternalOutput")
        dbg_q4 = nc.dram_tensor("dbg_q4", (4, 512), F32, kind="ExternalOutput")
        dbg_e = nc.dram_tensor("dbg_e", (128, 512), BF16, kind="ExternalOutput")
        dbg_wt = nc.dram_tensor("dbg_wt", (128, 512), BF16, kind="ExternalOutput")
        dbg_wtT = nc.dram_tensor("dbg_wtT", (128, 512), BF16, kind="ExternalOutput")

    x_d = nc.dram_tensor("x", (bl, 2, 128, HW), BF16, kind="ExternalInput")
    a_d = nc.dram_tensor("A", (128, 8, 128), BF16, kind="ExternalInput")
    onb_d = nc.dram_tensor("ONB", (128, 4, 128), FP16, kind="ExternalInput")
    scl_d = nc.dram_tensor("SCL", (128, 1), F32, kind="ExternalInput")
    bias_d = nc.dram_tensor("BIASB", (128, 1), F32, kind="ExternalInput")
    gs_d = nc.dram_tensor("GS", (128, 4), BF16, kind="ExternalInput")
    gb_d = nc.dram_tensor("GB", (4, 128), BF16, kind="ExternalInput")
    cw_d = nc.dram_tensor("CWD", (32, 256), F32, kind="ExternalInput")
    onz_d = nc.dram_tensor("ONZ", (128, 32), BF16, kind="ExternalInput")
    idt_d = nc.dram_tensor("IDT", (128, 128), BF16, kind="ExternalInput")
    out_d = nc.dram_tensor("out", (bl, 32, 256), F32, kind="ExternalOutput")

    with tile.TileContext(nc) as tc, ExitStack() as ctx:
        sb = ctx.enter_context(tc.tile_pool(name="sb", bufs=2))
        sbx = ctx.enter_context(tc.tile_pool(name="sbx", bufs=4))
        cp = ctx.enter_context(tc.tile_pool(name="consts", bufs=1))
        ps_xt = ctx.enter_context(tc.tile_pool(name="ps_xt", bufs=3, space="PSUM"))
        ps_big = ctx.enter_context(tc.tile_pool(name="ps_big", bufs=2, space="PSUM"))
        ps_d = ctx.enter_context(tc.tile_pool(name="ps_d", bufs=1, space="PSUM"))
        ps_wtt = ctx.enter_context(tc.tile_pool(name="ps_wtt", bufs=1, space="PSUM"))
        ps_wx = ctx.enter_context(tc.tile_pool(name="ps_wx", bufs=1, space="PSUM"))

        def c(shape, dram, tag, dt):
            t = cp.tile(shape, dt, tag=tag)
            nc.sync.dma_start(t[:], dram[:])
            return t

        # the two tiny consts the first transposes/evacs need, then the
        # unit-0 x loads (they gate the pipeline), then the other consts.
        idt_s = c([128, 128], idt_d, "idt", BF16)
        onz_s = c([128, 32], onz_d, "onz", BF16)
        xn0 = sbx.tile([128, 2, UNIT], BF16, tag="xn")
        nc.sync.dma_start(xn0[:, 0], x_d[0, 0, :, 0:UNIT])
        nc.sync.dma_start(xn0[:, 1], x_d[0, 1, :, 0:UNIT])

        scl_s = c([128, 1], scl_d, "scl", F32)
        bias_s = c([128, 1], bias_d, "bias", F32)
        gs_s = c([128, 4], gs_d, "gs", BF16)
        gb_s = c([4, 128], gb_d, "gb", BF16)
        cw_s = c([32, 256], cw_d, "cw", F32)
        onb_s = cp.tile([128, 4, 128], FP16, tag="onb")
        nc.sync.dma_start(onb_s[:], onb_d[:])
        a_s = cp.tile([128, 8, 128], BF16, tag="a")
        nc.sync.dma_start(a_s[:], a_d[:])

        pwx = {}
        pending_out = []

        def load_xn(u, xn=None, first=False):
            """Load x natural, fill xT's ones columns, and square xn for the
            PE |x|^2 reduction.  All run one unit ahead of stage(u)."""
            b_, uu = u // 2, u % 2
            t0 = uu * UNIT
            if xn is None:
                xn = sbx.tile([128, 2, UNIT], BF16, tag="xn")
                nc.sync.dma_start(xn[:, 0], x_d[b_, 0, :, t0:t0 + UNIT])
                nc.sync.dma_start(xn[:, 1], x_d[b_, 1, :, t0:t0 + UNIT])
            xT = sbx.tile([128, NCHUNK * XTW], BF16, tag="xT")
            xTv = xT[:].rearrange("p (j c) -> p j c", c=XTW)
            # col 256 = ones (mm2 col 256 accumulates wsum), col 257 = pad.
            nc.gpsimd.tensor_copy(
                xTv[:, :, 256:258],
                onz_s[:].rearrange("p (j c) -> p j c", c=2))
            # xqs[c, n] = xn0[c,n]^2 + xn1[c,n]^2 (fp16, DVE 2x): halves
            # the PE channel-reduction matmuls for |x|^2.  For the first
            # unit skip the pair-sum: 8 q-matmuls on the raw squares start
            # the pipeline ~3 us earlier (xq0 is ready right after the
            # first DMA half lands).
            xq0 = sb.tile([128, UNIT], FP16, tag="xq0")
            nc.vector.tensor_tensor(xq0[:], xn[:, 0], xn[:, 0], ALU.mult)
            xq1 = sb.tile([128, UNIT], FP16, tag="xq1")
            nc.vector.tensor_tensor(xq1[:], xn[:, 1], xn[:, 1], ALU.mult)
            if first:
                return xn, xT, (xq0, xq1)
            xq = sbx.tile([128, UNIT], FP16, tag="xq")
            nc.vector.tensor_tensor(xq[:], xq0[:], xq1[:], ALU.add)
            return xn, xT, xq

        def stage(u, prev, xn, xT, xq):
            """Emit A(u) interleaved with B(prev)."""
            b_, uu = u // 2, u % 2

            xTv = xT[:].rearrange("p (j c) -> p j c", c=XTW)
            st = dict(xT=xT, b=b_, uu=uu, u=u)
            psl2 = ps_big.tile([128, 512], F32, tag="big")
            st["psl2"] = psl2

            def mm1_part(i, start=False, stop=False):
                g, cc = divmod(i, 2)
                nc.tensor.matmul(
                    psl2[:, :],
                    a_s[:, cc * 4 + g, :],
                    xn[:, cc, g * 512:(g + 1) * 512],
                    start=start, stop=stop, skip_group_check=True,
                )

            def q_part(g, start=False, stop=False):
                # psl2[32g+k, n'] += sum_c xqs[c, 512g + n']  (exact fp32,
                # unscaled; exp applies scale_k per partition)
                if isinstance(xq, tuple):   # first unit: raw per-cc squares
                    for cc in (0, 1):
                        nc.tensor.matmul(
                            psl2[:, :],
                            onb_s[:, g, :],
                            xq[cc][:, g * 512:(g + 1) * 512],
                            start=(start and cc == 0),
                            stop=(stop and cc == 1), skip_group_check=True,
                        )
                else:
                    nc.tensor.matmul(
                        psl2[:, :],
                        onb_s[:, g, :],
                        xq[:, g * 512:(g + 1) * 512],
                        start=start, stop=stop, skip_group_check=True,
                    )

            def tgroup(j2):
                # PE transposes for both cc halves of 2 chunks
                xtp = ps_xt.tile([128, 512], BF16, tag="xt")
                for h in (0, 1):
                    j = 2 * j2 + h
                    for cc in (0, 1):
                        nc.tensor.transpose(
                            xtp[:, h * 256 + cc * 128:h * 256 + cc * 128 + 128],
                            xn[:, cc, j * 128:j * 128 + 128],
                            idt_s[:],
                        )
                # evacuate both chunks in one strided op
                dst = xTv[:, 2 * j2:2 * j2 + 2, 0:256]
                src = xtp[:].rearrange("p (h c) -> p h c", c=256)
                # unit 0: DVE is busy squaring x for the first |x|^2 pass,
                # so its evac tiles would stall the transpose rotation.
                if j2 in (0, 2, 3, 5, 6) or p is None:
                    nc.scalar.copy(dst, src)
                else:
                    nc.vector.tensor_copy(dst, src)

            p = prev  # may be None (first unit)

            # emit deferred batch-output stores: by now the STT that feeds
            # them has drained, so the DMA doesn't block the ACT sequencer.
            while pending_out:
                ob, outs = pending_out.pop(0)
                nc.scalar.dma_start(out_d[ob], outs[:])

            # |x|^2 channel-reduction matmuls accumulate straight into
            # psl2.  For unit 0 they are emitted late (xq(0) is still being
            # computed when PE starts; transposes/mm1 only need xn).
            if p is not None:
                for i in range(4):
                    q_part(i, start=(i == 0))
            tgroup(0)
            if p is not None:
                e = sb.tile([128, 512], BF16, tag="e")
                nc.scalar.activation(e[:], p["psl2"][:], ACTF.Exp,
                                     bias=bias_s[:], scale=scl_s[:])
            tgroup(1)
            mm1_part(0, start=(p is None))
            mm1_part(1)
            if p is not None:
                ps4 = ps_d.tile([4, 512], F32, tag="d")
                nc.tensor.matmul(ps4[:], gs_s[:], e[:])
            tgroup(2)
            mm1_part(2)
            if p is not None:
                r4 = sb.tile([4, 512], BF16, tag="r4")
                with nc.allow_low_precision(reason="1/d in bf16: per-token scale, cancels in out"):
                    nc.vector.reciprocal(r4[:], ps4[:])
            tgroup(3)
            mm1_part(3)
            if p is not None:
                pR = ps_big.tile([128, 512], F32, tag="big")
                nc.tensor.matmul(pR[:], gb_s[:], r4[:])
            tgroup(4)
            mm1_part(4)
            if p is not None:
                wt = sb.tile([128, 512], BF16, tag="wt")
                nc.vector.tensor_tensor(wt[:], e[:], pR[:], ALU.mult)
            tgroup(5)
            mm1_part(5)
            tgroup(6)
            mm1_part(6)
            if p is not None:
                if debug and p["u"] == 0:
                    nc.scalar.dma_start(dbg_xT[:], p["xT"][:])
                    nc.scalar.dma_start(dbg_e[:], e[:])
                    nc.scalar.dma_start(dbg_wt[:], wt[:])
                pwtT = ps_wtt.tile([128, 512], BF16, tag="wtt")
                for sl in range(4):
                    # transpose of the (128, 128) slice: column-block g of
                    # the result is wT for token-chunk j = 4*g + sl.
                    nc.tensor.transpose(
                        pwtT[:, 128 * sl:128 * sl + 128],
                        wt[:, 128 * sl:128 * sl + 128],
                        idt_s[:],
                    )
            if p is not None:
                wtTs = sb.tile([128, 512], BF16, tag="wtTs")
                nc.vector.tensor_copy(wtTs[:], pwtT[:])
                if debug and p["u"] == 0:
                    nc.scalar.dma_start(dbg_wtT[:], wtTs[:])
                emit_mm2(p, wtTs, 0, 12)
            tgroup(7)
            mm1_part(7, stop=(p is not None))
            if p is None:
                # unit 0: late q matmuls close the accumulation
                for i in range(4):
                    q_part(i, stop=(i == 3))
            if p is not None:
                emit_mm2(p, wtTs, 12, NCHUNK)
            # tail: prefetch + xbar + squares for u+1
            if u + 1 < units:
                nxt = load_xn(u + 1)
            else:
                nxt = (None, None, None)
            return st, nxt

        def emit_mm2(p, wtTs, j0=0, j1=NCHUNK):
            b_, uu, xT = p["b"], p["uu"], p["xT"]
            if uu == 0 and j0 == 0:
                pwx[b_] = ps_wx.tile([32, XTW], F32, tag="wx", name="pwx")
            for j in range(j0, j1):
                nc.tensor.matmul(
                    pwx[b_][:, 0:XTW],
                    wtTs[:, 128 * (j % 4) + 32 * (j // 4):128 * (j % 4) + 32 * (j // 4) + 32],
                    xT[:, XTW * j:XTW * j + XTW],
                    start=(uu == 0 and j == 0), stop=(uu == 1 and j == NCHUNK - 1),
                    skip_group_check=True,
                )
            if uu == 1 and j1 == NCHUNK:
                outs = sb.tile([32, 256], F32, tag="outs")
                nc.vector.scalar_tensor_tensor(
                    out=outs[:], in0=cw_s[:], scalar=pwx[b_][:, 256:257],
                    in1=pwx[b_][:, 0:256], op0=ALU.mult, op1=ALU.add,
                )
                pending_out.append((b_, outs))
                del pwx[b_]

        def stage_last(u, p, xn, xT, xq):
            """Final unit: close its logits early (q+mm1+fold front-loaded)
            and thread its softmax chain through the remaining transposes so
            the post-loop drain is short."""
            b_, uu = u // 2, u % 2
            xTv = xT[:].rearrange("p (j c) -> p j c", c=XTW)
            st = dict(xT=xT, b=b_, uu=uu, u=u)
            psl2 = ps_big.tile([128, 512], F32, tag="big")
            st["psl2"] = psl2

            def mm1_part(i, stop=False):
                g, cc = divmod(i, 2)
                nc.tensor.matmul(
                    psl2[:, :], a_s[:, cc * 4 + g, :],
                    xn[:, cc, g * 512:(g + 1) * 512],
                    start=False, stop=stop, skip_group_check=True,
                )

            def tgroup(j2):
                xtp = ps_xt.tile([128, 512], BF16, tag="xt")
                for h in (0, 1):
                    j = 2 * j2 + h
                    for cc in (0, 1):
                        nc.tensor.transpose(
                            xtp[:, h * 256 + cc * 128:h * 256 + cc * 128 + 128],
                            xn[:, cc, j * 128:j * 128 + 128], idt_s[:],
                        )
                dst = xTv[:, 2 * j2:2 * j2 + 2, 0:256]
                srcv = xtp[:].rearrange("p (h c) -> p h c", c=256)
                if j2 in (0, 2, 3, 5, 6):
                    nc.scalar.copy(dst, srcv)
                else:
                    nc.vector.tensor_copy(dst, srcv)

            while pending_out:
                ob, outs = pending_out.pop(0)
                nc.scalar.dma_start(out_d[ob], outs[:])

            e_p = sb.tile([128, 512], BF16, tag="e")
            nc.scalar.activation(e_p[:], p["psl2"][:], ACTF.Exp,
                                 bias=bias_s[:], scale=scl_s[:])
            for g in range(4):
                nc.tensor.matmul(
                    psl2[:, :], onb_s[:, g, :], xq[:, g * 512:(g + 1) * 512],
                    start=(g == 0), stop=False, skip_group_check=True,
                )
            for i in range(4):
                mm1_part(i)
            ps4_p = ps_d.tile([4, 512], F32, tag="d")
            nc.tensor.matmul(ps4_p[:], gs_s[:], e_p[:])
            for i in range(4, 8):
                mm1_part(i, stop=(i == 7))
            r4_p = sb.tile([4, 512], BF16, tag="r4")
            with nc.allow_low_precision(reason="1/d in bf16: per-token scale, cancels in out"):
                nc.vector.reciprocal(r4_p[:], ps4_p[:])
            e_u = sb.tile([128, 512], BF16, tag="e")
            nc.scalar.activation(e_u[:], psl2[:], ACTF.Exp,
                                 bias=bias_s[:], scale=scl_s[:])
            tgroup(0)
            tgroup(1)
            pR_p = ps_big.tile([128, 512], F32, tag="big")
            nc.tensor.matmul(pR_p[:], gb_s[:], r4_p[:])
            tgroup(2)
            wt_p = sb.tile([128, 512], BF16, tag="wt")
            nc.vector.tensor_tensor(wt_p[:], e_p[:], pR_p[:], ALU.mult)
            ps4_u = ps_d.tile([4, 512], F32, tag="d")
            nc.tensor.matmul(ps4_u[:], gs_s[:], e_u[:])
            tgroup(3)
            r4_u = sb.tile([4, 512], BF16, tag="r4")
            with nc.allow_low_precision(reason="1/d in bf16: per-token scale, cancels in out"):
                nc.vector.reciprocal(r4_u[:], ps4_u[:])
            pwtT_p = ps_wtt.tile([128, 512], BF16, tag="wtt")
            for sl in range(4):
                nc.tensor.transpose(
                    pwtT_p[:, 128 * sl:128 * sl + 128],
                    wt_p[:, 128 * sl:128 * sl + 128], idt_s[:],
                )
            tgroup(4)
            pR_u = ps_big.tile([128, 512], F32, tag="big")
            nc.tensor.matmul(pR_u[:], gb_s[:], r4_u[:])
            wtTs_p = sb.tile([128, 512], BF16, tag="wtTs")
            nc.vector.tensor_copy(wtTs_p[:], pwtT_p[:])
            tgroup(5)
            wt_u = sb.tile([128, 512], BF16, tag="wt")
            nc.vector.tensor_tensor(wt_u[:], e_u[:], pR_u[:], ALU.mult)
            emit_mm2(p, wtTs_p)
            tgroup(6)
            tgroup(7)
            return st, wt_u

        cur = load_xn(0, xn=xn0, first=True)
        prev = None
        for u in range(units - 1):
            prev, cur = stage(u, prev, *cur)
        prev, wt_last = stage_last(units - 1, prev, *cur)
        # epilogue: only the tail of the last unit's chain remains
        pwtT = ps_wtt.tile([128, 512], BF16, tag="wtt")
        for sl in range(4):
            nc.tensor.transpose(
                pwtT[:, 128 * sl:128 * sl + 128],
                wt_last[:, 128 * sl:128 * sl + 128],
                idt_s[:],
            )
        wtTs = sb.tile([128, 512], BF16, tag="wtTs")
        nc.vector.tensor_copy(wtTs[:], pwtT[:])
        emit_mm2(prev, wtTs)
        while pending_out:
            ob, outs = pending_out.pop(0)
            nc.scalar.dma_start(out_d[ob], outs[:])

    nc.finalize()
    return nc


def host_constants(codewords, scale):
    cw = np.asarray(codewords, dtype=np.float32)
    sc = np.asarray(scale, dtype=np.float32)
    c_sq = (cw.astype(np.float64) ** 2).sum(-1).astype(np.float32)

    A = np.zeros((2, 4, 128, 128), np.float32)
    for cc in range(2):
        blk = -2.0 * cw[:, cc * 128:(cc + 1) * 128].T
        for g in range(4):
            A[cc, g, :, 32 * g:32 * g + 32] = blk

    ONB = np.zeros((128, 4, 128), np.float32)
    SCL = np.zeros((128, 1), np.float32)
    BIASB = np.zeros((128, 1), np.float32)
    GS = np.zeros((128, 4), np.float32)
    GB = np.zeros((4, 128), np.float32)
    for g in range(4):
        ONB[:, g, 32 * g:32 * g + 32] = 1.0
        SCL[32 * g:32 * g + 32, 0] = sc
        BIASB[32 * g:32 * g + 32, 0] = sc * c_sq
        GS[32 * g:32 * g + 32, g] = 1.0
        GB[g, 32 * g:32 * g + 32] = 1.0

    bf = ml_dtypes.bfloat16
    return {
        "A": np.ascontiguousarray(A.transpose(2, 0, 1, 3).reshape(128, 8, 128)).astype(bf),
        "ONB": ONB.astype(np.float16), "SCL": SCL,
        "BIASB": BIASB, "GS": GS.astype(bf), "GB": GB.astype(bf),
        "CWD": np.ascontiguousarray(-cw),
        "ONZ": np.tile(np.array([1.0, 0.0], bf), (128, 16)),
        "IDT": np.eye(128, dtype=bf),
    }


def make_in_maps(x, codewords, scale):
    consts = host_constants(codewords, scale)
    xb = np.asarray(x).astype(ml_dtypes.bfloat16)
    xs = xb.reshape(B, 2, 128, HW)
    in_maps = []
    for i in range(N_CORES):
        m = dict(consts)
        m["x"] = np.ascontiguousarray(xs[BL * i:BL * (i + 1)])
        in_maps.append(m)
    return in_maps


_CACHE = {}


def kernel(x, codewords, scale):
    if "nc" not in _CACHE:
        _CACHE["nc"] = build_module()
    nc = _CACHE["nc"]
    in_maps = make_in_maps(x, codewords, scale)
    res = run_bass_kernel_spmd(nc, in_maps, list(range(N_CORES)))
    out = np.concatenate([r["out"] for r in res.results], axis=0)
    return out.astype(np.float32)
